# revision 27
# baseline (speedup 1.0000x reference)
"""Causal attention kernel for Trainium2, 8 NeuronCores, sequence-parallel.

Reference computation (T=4096, D=1024, fp32):
    q = x @ Wqk; logits = q @ x.T (causal masked); attn = softmax(logits)
    out = (attn @ x) @ Wov

Causal-balanced sharding: global 128-row query tiles i = 0..31 need
keys 0..128(i+1), i.e. w_i = i//4 + 1 key slots of 512. Core c owns
tiles {c, 8+c, 16+c, 24+c} (local m = 0..3, global g = 8m + c), and the
SPMD program gives local tile m a fixed capacity of 2m+2 key slots
(widths 1024/2048/3072/4096). Every core's needs fit exactly:
  c in 0..3: tile m needs 2m+1 slots -> slot 2m is ragged-diagonal,
             slot 2m+1 is fully masked.
  c in 4..7: tile m needs 2m+2 slots -> slot 2m fully visible,
             slot 2m+1 ragged-diagonal.
Keys stay in NATURAL order and are identical on all cores; only the
query-row selection (xqt columns) and two additive mask tiles differ
per core.  maskA applies at slot 2m, maskB at slot 2m+1, for every m:
  c < 4:  maskA = tri(offset 128c),      maskB = all -60000
  c >= 4: maskA = 0,                     maskB = tri(offset 128(c-4))
This cuts score and AV matmul work to 62.5% of the dense version while
keeping one identical instruction stream on all 8 cores.

Precision: fp16 operands (x, Wqk, Wov, q, attn, o1) with fp32 PSUM
accumulation and fp32 softmax stats; masked-out logits get -60000
(fp16-representable; exp underflows to exactly 0). Host-validated
rel_err ~3e-3 (limit 2e-2).
"""

import sys

sys.path.insert(0, "/opt/trn_rl_repo")

import numpy as np

import concourse.tile as tile
from concourse import bacc, mybir
from concourse.bass_utils import run_bass_kernel_spmd

T = 4096
D = 1024
NCORES = 8
RQ = T // NCORES  # 512 query rows per core
NKB = T // 512  # 8 key slots of 512
KC = D // 128  # 8 contraction chunks
NMT = RQ // 128  # 4 query-row tiles per core
CAP = [2 * m + 2 for m in range(NMT)]  # key-slot capacity per local tile
NEG = -60000.0

f32 = mybir.dt.float32
f16 = mybir.dt.float16


def _build_nc():
    nc = bacc.Bacc(
        "TRN2", target_bir_lowering=False, debug=False, num_devices=NCORES
    )

    xqt_d = nc.dram_tensor("xqt", [D, RQ], f16, kind="ExternalInput").ap()
    xtp_d = nc.dram_tensor("xtp", [D, T], f16, kind="ExternalInput").ap()
    xp_d = nc.dram_tensor("xp", [T, D], f16, kind="ExternalInput").ap()
    wqk_d = nc.dram_tensor("wqk", [D, D], f16, kind="ExternalInput").ap()
    wov_d = nc.dram_tensor("wov", [D, D], f16, kind="ExternalInput").ap()
    maska_d = nc.dram_tensor("maska", [128, 512], f16, kind="ExternalInput").ap()
    maskb_d = nc.dram_tensor("maskb", [128, 512], f16, kind="ExternalInput").ap()
    out_d = nc.dram_tensor("out", [RQ, D], f32, kind="ExternalOutput").ap()

    with tile.TileContext(nc) as tc:
        # stack allocator: allocate in order of decreasing lifetime
        consts = tc.alloc_tile_pool(name="consts", bufs=1)
        o1_pool = tc.alloc_tile_pool(name="o1pool", bufs=1)
        pt_pool = tc.alloc_tile_pool(name="ptpool", bufs=1)
        xp_pool = tc.alloc_tile_pool(name="xppool", bufs=1)
        s_pool = tc.alloc_tile_pool(name="spool", bufs=1)
        p_pool = tc.alloc_tile_pool(name="ppool", bufs=3)
        qt_pool = tc.alloc_tile_pool(name="qtpool", bufs=1)

        # constants: masks + stats scratch
        smalls = consts.tile([128, 64], f32, name="smalls")
        negmax = smalls[:, 0:NMT]
        lsum = smalls[:, 4:8]
        recip = smalls[:, 8:12]
        mpart = smalls[:, 12:44]  # [m * NKB + kb]
        lq = smalls[:, 44:60]  # [m * 4 + ch]
        maska = consts.tile([128, 512], f16, name="maska")
        maskb = consts.tile([128, 512], f16, name="maskb")

        # long-lived big tiles
        o1t_sb = o1_pool.tile([128, KC * RQ], f16, name="o1t_sb")
        pt_tiles = [
            pt_pool.tile([128, 8 * (m + 1) * 128], f16, name=f"pt_m{m}")
            for m in range(NMT)
        ]
        pt_views = [
            ptm.rearrange("p (kcc q) -> p kcc q", kcc=8 * (m + 1))
            for m, ptm in enumerate(pt_tiles)
        ]
        # 8 separate xp tiles (512 keys each) so E's RAW deps are per-chunk
        xp_tiles = [
            xp_pool.tile([128, 4 * D], f16, name=f"xp_{j}") for j in range(8)
        ]
        xp_views = [t.rearrange("p (kc n) -> p kc n", kc=4) for t in xp_tiles]
        s_tiles = [
            s_pool.tile([128, 1024 * (m + 1)], f32, name=f"s_m{m}")
            for m in range(NMT)
        ]
        qt_sb = qt_pool.tile([128, KC * RQ], f16, name="qt_sb")

        # right-side stack: lifetime [A..B] overlaps the left stack's
        # wov pool [A-end..F], so it gets its own stack side
        xtstream = tc.alloc_tile_pool(name="xtstream", bufs=5, side="right")

        xtp_src = xtp_d.rearrange("p (kb n) -> p kb n", kb=NKB)
        xt_views = []

        def issue_xt():
            kb = len(xt_views)
            xt = xtstream.tile([128, KC * 512], f16, name="xt", tag="xt")
            xt_v = xt.rearrange("p (kc n) -> p kc n", kc=KC)
            nc.sync.dma_start(
                xt_v, xtp_src[:, kb, :].rearrange("(kc p) n -> p kc n", p=128)
            )
            xt_views.append(xt_v)

        # ---- Phase A: qT = (xq @ Wqk)^T -> [D, RQ] fp16 ------------------
        # wqk streamed in 256-col pairs (512B descriptors, no small-desc
        # penalty); each pair feeds two 8-matmul chains so the DMA stream
        # stays ahead of the PE. xt/mask loads ride the spare bandwidth.
        with (
            tc.tile_pool(name="apool", bufs=1) as apool,
            tc.tile_pool(name="wqkstream", bufs=2) as wqkstream,
            tc.tile_pool(name="psA", bufs=2, space="PSUM") as psA,
        ):
            xqt_sb = apool.tile([128, KC * RQ], f16, name="xqt_sb")
            xqt_v = xqt_sb.rearrange("p (kc n) -> p kc n", kc=KC)
            xqt_src = xqt_d.rearrange("(kc p) n -> p kc n", p=128)
            for j in range(4):
                wqk_blk = wqkstream.tile(
                    [128, KC * 256], f16, name="wqk_blk", tag="wq"
                )
                wqk_v = wqk_blk.rearrange("p (kc n) -> p kc n", kc=KC)
                nc.sync.dma_start(
                    wqk_v,
                    wqk_d[:, j * 256 : (j + 1) * 256].rearrange(
                        "(kc p) n -> p kc n", p=128
                    ),
                )
                if j == 0:
                    nc.sync.dma_start(xqt_v[:, 0:4, :], xqt_src[:, 0:4, :])
                    nc.sync.dma_start(xqt_v[:, 4:8, :], xqt_src[:, 4:8, :])
                elif j == 1:
                    nc.sync.dma_start(maska, maska_d)
                    nc.sync.dma_start(maskb, maskb_d)
                elif j == 2:
                    issue_xt()
                else:
                    issue_xt()
                    issue_xt()
                for sub in range(2):
                    mtd = 2 * j + sub
                    ps = psA.tile([128, RQ], f32, name="ps_qt")
                    for kc in range(KC):
                        nc.tensor.matmul(
                            ps[:],
                            wqk_v[:, kc, sub * 128 : (sub + 1) * 128],
                            xqt_v[:, kc, :],
                            start=(kc == 0),
                            stop=(kc == KC - 1),
                        )
                    nc.vector.tensor_copy(
                        qt_sb[:, mtd * RQ : (mtd + 1) * RQ], ps[:]
                    )

        # wov loads reuse the SBUF apool just freed; queued here so they
        # land after xt0-2 but before the rest of the B stream
        wov_pool = tc.alloc_tile_pool(name="wovstream", bufs=1)
        wov_tiles = []
        for nb in range(2):
            wov_blk = wov_pool.tile([128, KC * 512], f16, name=f"wov{nb}")
            wov_tiles.append(wov_blk)
            wv = wov_blk.rearrange("p (kc n) -> p kc n", kc=KC)
            src = wov_d[:, nb * 512 : (nb + 1) * 512].rearrange(
                "(kc p) n -> p kc n", p=128
            )
            # half-size pieces: keeps the exclusive DMA resource fine-grained
            # so latency-critical transposes are not stuck behind them
            nc.sync.dma_start(wv[:, 0:4, :], src[:, 0:4, :])
            nc.sync.dma_start(wv[:, 4:8, :], src[:, 4:8, :])

        # ---- Phase B: ragged scores + fused softmax prep -----------------
        # slot kb serves local tiles m with CAP[m] > kb; masks at slots
        # 2m (maskA) and 2m+1 (maskB); exp+transpose issued per tile as
        # soon as its last slot completes. xp/wov loads ride the late-B
        # DMA shadow, in time for phases E/F.
        with tc.tile_pool(name="psB", bufs=3, space="PSUM") as psB:
            for kb in range(NKB):
                if kb + 3 < NKB:
                    issue_xt()
                if kb >= 4:
                    # xp rides the late-B DMA shadow in half-chunk pieces
                    # (256 keys each) so transposes interleave promptly
                    for h in range(4 * (kb - 4), 4 * (kb - 3)):
                        jj, hh = h // 2, h % 2
                        nc.sync.dma_start(
                            xp_views[jj][:, 2 * hh : 2 * (hh + 1), :],
                            xp_d[
                                jj * 512 + hh * 256 : jj * 512 + (hh + 1) * 256, :
                            ].rearrange("(kc p) n -> p kc n", p=128),
                        )
                xt_v = xt_views[kb]
                for m in range(NMT):
                    if CAP[m] <= kb:
                        continue
                    ps = psB.tile([128, 512], f32, name="ps_s")
                    for kc in range(KC):
                        nc.tensor.matmul(
                            ps[:],
                            qt_sb[:, kc * RQ + m * 128 : kc * RQ + (m + 1) * 128],
                            xt_v[:, kc, :],
                            start=(kc == 0),
                            stop=(kc == KC - 1),
                        )
                    dst = s_tiles[m][:, kb * 512 : (kb + 1) * 512]
                    if kb == 2 * m:
                        nc.vector.tensor_add(dst, ps[:], maska[:])
                    elif kb == 2 * m + 1:
                        nc.vector.tensor_add(dst, ps[:], maskb[:])
                    else:
                        nc.vector.tensor_copy(dst, ps[:])
                    nc.vector.tensor_reduce(
                        mpart[:, m * NKB + kb : m * NKB + kb + 1],
                        dst,
                        axis=mybir.AxisListType.X,
                        op=mybir.AluOpType.max,
                    )
                    if kb == CAP[m] - 1:
                        # tile m complete: finalize stats, exp, transpose
                        nc.vector.tensor_reduce(
                            negmax[:, m : m + 1],
                            mpart[:, m * NKB : m * NKB + CAP[m]],
                            axis=mybir.AxisListType.X,
                            op=mybir.AluOpType.max,
                            negate=True,
                        )
                        for ch in range(m + 1):
                            p_q = p_pool.tile([128, 1024], f16, name="p_q", tag="pq")
                            nc.scalar.activation(
                                p_q[:],
                                s_tiles[m][:, ch * 1024 : (ch + 1) * 1024],
                                mybir.ActivationFunctionType.Exp,
                                bias=negmax[:, m : m + 1],
                                scale=1.0,
                                accum_out=lq[:, m * 4 + ch : m * 4 + ch + 1],
                            )
                            nc.scalar.dma_start_transpose(
                                pt_views[m][:, ch * 8 : (ch + 1) * 8, :], p_q[:]
                            )
                        # lsum on the idle Pool engine: keeps this exp-gated
                        # chain out of the DVE FIFO, which must stay clear
                        # for E's PSUM copies (recip is emitted later, in
                        # the E/F loop, for the same reason)
                        nc.gpsimd.tensor_copy(
                            lsum[:, m : m + 1], lq[:, m * 4 : m * 4 + 1]
                        )
                        for ch in range(1, m + 1):
                            nc.gpsimd.tensor_add(
                                lsum[:, m : m + 1],
                                lsum[:, m : m + 1],
                                lq[:, m * 4 + ch : m * 4 + ch + 1],
                            )
        xtstream.release()

        # ---- Phases E+F interleaved per row tile -------------------------
        # E(m): o1T[:, m] = sum_k x[k,:]^T P[m,k]^T (ragged contraction),
        # then immediately F(m): out[m] = (o1[m] @ Wov) * recip[m].
        # Keeps the PE stream gapless across the phase boundary; E(m<3)
        # also hides the exp/transpose tail of tile 3.
        with (
            tc.tile_pool(name="psE", bufs=2, space="PSUM") as psE,
            tc.tile_pool(name="psF", bufs=2, space="PSUM") as psF,
            tc.tile_pool(name="outp", bufs=3) as outp,
        ):
            for m in range(NMT):
                nk = 8 * (m + 1)
                for mtd in range(KC):
                    ps = psE.tile([128, 128], f32, name="ps_av")
                    for kcc in range(nk):
                        nc.tensor.matmul(
                            ps[:],
                            xp_views[kcc // 4][:, kcc % 4, mtd * 128 : (mtd + 1) * 128],
                            pt_views[m][:, kcc, :],
                            start=(kcc == 0),
                            stop=(kcc == nk - 1),
                        )
                    nc.vector.tensor_copy(
                        o1t_sb[:, mtd * RQ + m * 128 : mtd * RQ + (m + 1) * 128],
                        ps[:],
                    )
                nc.vector.reciprocal(recip[:, m : m + 1], lsum[:, m : m + 1])
                for nb in range(2):
                    ps = psF.tile([128, 512], f32, name="ps_o")
                    for kc in range(KC):
                        nc.tensor.matmul(
                            ps[:],
                            o1t_sb[:, kc * RQ + m * 128 : kc * RQ + (m + 1) * 128],
                            wov_tiles[nb][:, kc * 512 : (kc + 1) * 512],
                            start=(kc == 0),
                            stop=(kc == KC - 1),
                        )
                    ob = outp.tile([128, 512], f32, name="ob")
                    nc.vector.tensor_scalar_mul(
                        ob[:], ps[:], recip[:, m : m + 1]
                    )
                    nc.sync.dma_start(
                        out_d[m * 128 : (m + 1) * 128, nb * 512 : (nb + 1) * 512],
                        ob[:],
                    )

        wov_pool.release()
        qt_pool.release()
        p_pool.release()
        s_pool.release()
        xp_pool.release()
        pt_pool.release()
        o1_pool.release()
        consts.release()

    nc.compile()
    return nc


_NC_CACHE = {}


def _get_nc():
    if "nc" not in _NC_CACHE:
        _NC_CACHE["nc"] = _build_nc()
    return _NC_CACHE["nc"]


def _prep_in_maps(x, Wqk, Wov):
    x = np.ascontiguousarray(np.asarray(x), dtype=np.float32)
    Wqk = np.ascontiguousarray(np.asarray(Wqk), dtype=np.float32)
    Wov = np.ascontiguousarray(np.asarray(Wov), dtype=np.float32)
    x16 = x.astype(np.float16)
    xtp = np.ascontiguousarray(x16.T)  # [D, T] natural key order
    wqk16 = Wqk.astype(np.float16)
    wov16 = Wov.astype(np.float16)

    p = np.arange(128)[:, None]
    col = np.arange(512)[None, :]

    in_maps = []
    for c in range(NCORES):
        rows = np.concatenate(
            [np.arange(128 * (8 * m + c), 128 * (8 * m + c) + 128) for m in range(NMT)]
        )
        xqt = np.ascontiguousarray(x16[rows, :].T)  # [D, RQ]
        if c < 4:
            maska = np.where(col <= 128 * c + p, 0.0, NEG).astype(np.float16)
            maskb = np.full((128, 512), NEG, np.float16)
        else:
            maska = np.zeros((128, 512), np.float16)
            maskb = np.where(col <= 128 * (c - 4) + p, 0.0, NEG).astype(np.float16)
        in_maps.append(
            {
                "xqt": xqt,
                "xtp": xtp,
                "xp": x16,
                "wqk": wqk16,
                "wov": wov16,
                "maska": np.ascontiguousarray(maska),
                "maskb": np.ascontiguousarray(maskb),
            }
        )
    return in_maps


def run(x, Wqk, Wov, **spmd_kwargs):
    """Full pipeline; returns (output [T, D] fp32, BassKernelResults)."""
    import time

    nc = _get_nc()
    in_maps = _prep_in_maps(x, Wqk, Wov)
    try:
        res = run_bass_kernel_spmd(
            nc, in_maps, core_ids=list(range(NCORES)), **spmd_kwargs
        )
    except Exception:
        # a prior crashed execution can leave a core transiently
        # unrecoverable; the runtime resets it — retry once
        time.sleep(10)
        res = run_bass_kernel_spmd(
            nc, in_maps, core_ids=list(range(NCORES)), **spmd_kwargs
        )
    out = np.empty((T, D), np.float32)
    for c in range(NCORES):
        oc = res.results[c]["out"]
        for m in range(NMT):
            g = 8 * m + c
            out[128 * g : 128 * (g + 1), :] = oc[128 * m : 128 * (m + 1), :]
    return out, res


def kernel(x, Wqk, Wov):
    out, _ = run(x, Wqk, Wov)
    return out


# revision 39
# speedup vs baseline: 1.2567x; 1.2567x over previous
"""Causal attention kernel for Trainium2, 8 NeuronCores, sequence-parallel.

Reference computation (T=4096, D=1024, fp32):
    q = x @ Wqk; logits = q @ x.T (causal masked); attn = softmax(logits)
    out = (attn @ x) @ Wov

Causal-balanced sharding: global 128-row query tiles i = 0..31 need
keys 0..128(i+1), i.e. w_i = i//4 + 1 key slots of 512. Core c owns
tiles {c, 8+c, 16+c, 24+c} (local m = 0..3, global g = 8m + c), and the
SPMD program gives local tile m a fixed capacity of 2m+2 key slots
(widths 1024/2048/3072/4096). Every core's needs fit exactly:
  c in 0..3: tile m needs 2m+1 slots -> slot 2m is ragged-diagonal,
             slot 2m+1 is fully masked.
  c in 4..7: tile m needs 2m+2 slots -> slot 2m fully visible,
             slot 2m+1 ragged-diagonal.
Keys stay in NATURAL order and are identical on all cores; only the
query-row selection (xqt columns) and two additive mask tiles differ
per core.  maskA applies at slot 2m, maskB at slot 2m+1, for every m:
  c < 4:  maskA = tri(offset 128c),      maskB = all -60000
  c >= 4: maskA = 0,                     maskB = tri(offset 128(c-4))
This cuts score and AV matmul work to 62.5% of the dense version while
keeping one identical instruction stream on all 8 cores.

Precision: fp16 operands (x, Wqk, Wov, q, attn, o1) with fp32 PSUM
accumulation and fp32 softmax stats; masked-out logits get -60000
(fp16-representable; exp underflows to exactly 0). Host-validated
rel_err ~3e-3 (limit 2e-2).
"""

import sys

sys.path.insert(0, "/opt/trn_rl_repo")

import numpy as np

import concourse.tile as tile
from concourse import bacc, mybir
from concourse.bass_utils import run_bass_kernel_spmd

T = 4096
D = 1024
NCORES = 8
RQ = T // NCORES  # 512 query rows per core
NKB = T // 512  # 8 key slots of 512
KC = D // 128  # 8 contraction chunks
NMT = RQ // 128  # 4 query-row tiles per core
CAP = [2 * m + 2 for m in range(NMT)]  # key-slot capacity per local tile
NEG = -60000.0

f32 = mybir.dt.float32
f16 = mybir.dt.float16


def _build_nc():
    nc = bacc.Bacc(
        "TRN2", target_bir_lowering=False, debug=False, num_devices=NCORES
    )

    xqt_d = nc.dram_tensor("xqt", [D, RQ], f16, kind="ExternalInput").ap()
    xtp_d = nc.dram_tensor("xtp", [D, T], f16, kind="ExternalInput").ap()
    xp_d = nc.dram_tensor("xp", [T, D], f16, kind="ExternalInput").ap()
    wqk_d = nc.dram_tensor("wqk", [D, D], f16, kind="ExternalInput").ap()
    wov_d = nc.dram_tensor("wov", [D, D], f16, kind="ExternalInput").ap()
    maska_d = nc.dram_tensor("maska", [128, 512], f16, kind="ExternalInput").ap()
    maskb_d = nc.dram_tensor("maskb", [128, 512], f16, kind="ExternalInput").ap()
    ident_d = nc.dram_tensor("ident", [128, 128], f16, kind="ExternalInput").ap()
    out_d = nc.dram_tensor("out", [RQ, D], f32, kind="ExternalOutput").ap()

    with tile.TileContext(nc) as tc:
        # stack allocator: allocate in order of decreasing lifetime
        consts = tc.alloc_tile_pool(name="consts", bufs=1)
        o1_pool = tc.alloc_tile_pool(name="o1pool", bufs=1)
        pt_pool = tc.alloc_tile_pool(name="ptpool", bufs=1)
        xp_pool = tc.alloc_tile_pool(name="xppool", bufs=1)
        s_pool = tc.alloc_tile_pool(name="spool", bufs=1)
        p_pool = tc.alloc_tile_pool(name="ppool", bufs=4)
        qt_pool = tc.alloc_tile_pool(name="qtpool", bufs=1)
        # PSUM staging for PE transposes of attn chunks; outlives psB
        psT = tc.alloc_tile_pool(name="psT", bufs=2, space="PSUM")

        # constants: masks + stats scratch
        smalls = consts.tile([128, 64], f32, name="smalls")
        negmax = smalls[:, 0:NMT]
        lsum = smalls[:, 4:8]
        recip = smalls[:, 8:12]
        mpart = smalls[:, 12:44]  # [m * NKB + kb]
        lq = smalls[:, 44:60]  # [m * 4 + ch]
        maska = consts.tile([128, 512], f16, name="maska")
        maskb = consts.tile([128, 512], f16, name="maskb")
        ident = consts.tile([128, 128], f16, name="ident")

        # long-lived big tiles
        o1t_sb = o1_pool.tile([128, KC * RQ], f16, name="o1t_sb")
        pt_tiles = [
            pt_pool.tile([128, 8 * (m + 1) * 128], f16, name=f"pt_m{m}")
            for m in range(NMT)
        ]
        pt_views = [
            ptm.rearrange("p (kcc q) -> p kcc q", kcc=8 * (m + 1))
            for m, ptm in enumerate(pt_tiles)
        ]
        # 8 separate xp tiles (512 keys each) so E's RAW deps are per-chunk
        xp_tiles = [
            xp_pool.tile([128, 4 * D], f16, name=f"xp_{j}") for j in range(8)
        ]
        xp_views = [t.rearrange("p (kc n) -> p kc n", kc=4) for t in xp_tiles]
        s_tiles = [
            s_pool.tile([128, 1024 * (m + 1)], f32, name=f"s_m{m}")
            for m in range(NMT)
        ]
        qt_sb = qt_pool.tile([128, KC * RQ], f16, name="qt_sb")

        # right-side stack: lifetime [A..B] overlaps the left stack's
        # wov pool [A-end..F], so it gets its own stack side
        xtstream = tc.alloc_tile_pool(name="xtstream", bufs=5, side="right")

        xtp_src = xtp_d.rearrange("p (kb n) -> p kb n", kb=NKB)
        xt_views = []

        def issue_xt():
            kb = len(xt_views)
            xt = xtstream.tile([128, KC * 512], f16, name="xt", tag="xt")
            xt_v = xt.rearrange("p (kc n) -> p kc n", kc=KC)
            nc.sync.dma_start(
                xt_v, xtp_src[:, kb, :].rearrange("(kc p) n -> p kc n", p=128)
            )
            xt_views.append(xt_v)

        # ---- Phase A: qT = (xq @ Wqk)^T -> [D, RQ] fp16 ------------------
        # wqk streamed in 256-col pairs (512B descriptors, no small-desc
        # penalty); each pair feeds two 8-matmul chains so the DMA stream
        # stays ahead of the PE. xt/mask loads ride the spare bandwidth.
        with (
            tc.tile_pool(name="apool", bufs=1) as apool,
            tc.tile_pool(name="wqkstream", bufs=2) as wqkstream,
            tc.tile_pool(name="psA", bufs=2, space="PSUM") as psA,
        ):
            xqt_sb = apool.tile([128, KC * RQ], f16, name="xqt_sb")
            xqt_v = xqt_sb.rearrange("p (kc n) -> p kc n", kc=KC)
            xqt_src = xqt_d.rearrange("(kc p) n -> p kc n", p=128)
            for j in range(4):
                wqk_blk = wqkstream.tile(
                    [128, KC * 256], f16, name="wqk_blk", tag="wq"
                )
                wqk_v = wqk_blk.rearrange("p (kc n) -> p kc n", kc=KC)
                nc.sync.dma_start(
                    wqk_v,
                    wqk_d[:, j * 256 : (j + 1) * 256].rearrange(
                        "(kc p) n -> p kc n", p=128
                    ),
                )
                if j == 0:
                    nc.sync.dma_start(xqt_v[:, 0:4, :], xqt_src[:, 0:4, :])
                    nc.sync.dma_start(xqt_v[:, 4:8, :], xqt_src[:, 4:8, :])
                elif j == 1:
                    nc.sync.dma_start(maska, maska_d)
                    nc.sync.dma_start(maskb, maskb_d)
                    nc.sync.dma_start(ident, ident_d)
                elif j == 2:
                    issue_xt()
                else:
                    issue_xt()
                    issue_xt()
                for sub in range(2):
                    mtd = 2 * j + sub
                    ps = psA.tile([128, RQ], f32, name="ps_qt")
                    for kc in range(KC):
                        nc.tensor.matmul(
                            ps[:],
                            wqk_v[:, kc, sub * 128 : (sub + 1) * 128],
                            xqt_v[:, kc, :],
                            start=(kc == 0),
                            stop=(kc == KC - 1),
                        )
                    nc.vector.tensor_copy(
                        qt_sb[:, mtd * RQ : (mtd + 1) * RQ], ps[:]
                    )

        # wov loads reuse the SBUF apool just freed; queued here so they
        # land after xt0-2 but before the rest of the B stream
        wov_pool = tc.alloc_tile_pool(name="wovstream", bufs=1)
        wov_tiles = []
        for nb in range(2):
            wov_blk = wov_pool.tile([128, KC * 512], f16, name=f"wov{nb}")
            wov_tiles.append(wov_blk)
            wv = wov_blk.rearrange("p (kc n) -> p kc n", kc=KC)
            src = wov_d[:, nb * 512 : (nb + 1) * 512].rearrange(
                "(kc p) n -> p kc n", p=128
            )
            # half-size pieces: keeps the exclusive DMA resource fine-grained
            # so latency-critical transposes are not stuck behind them
            nc.sync.dma_start(wv[:, 0:4, :], src[:, 0:4, :])
            nc.sync.dma_start(wv[:, 4:8, :], src[:, 4:8, :])

        # ---- Phase B: ragged scores + fused softmax prep -----------------
        # slot kb serves local tiles m with CAP[m] > kb; masks at slots
        # 2m (maskA) and 2m+1 (maskB); exp+transpose issued per tile as
        # soon as its last slot completes. xp/wov loads ride the late-B
        # DMA shadow, in time for phases E/F.
        p_chunks = [[] for _ in range(NMT)]

        def emit_pt_transposes(m):
            # PE-transpose tile m's attn chunks into pt layout; emitted a
            # couple of key slots after tile m's exp so the PE never waits
            # on the ACT engine. Groups of 4 kcc share one PSUM stage and
            # one DVE copy.
            for ch in range(m + 1):
                p_q = p_chunks[m][ch]
                for g in range(2):
                    pst = psT.tile([128, 512], f16, name="pst", tag="pst")
                    for q4 in range(4):
                        col = (g * 4 + q4) * 128
                        nc.tensor.transpose(
                            pst[:, q4 * 128 : (q4 + 1) * 128],
                            p_q[:, col : col + 128],
                            ident[:],
                        )
                    base = (ch * 8 + g * 4) * 128
                    nc.vector.tensor_copy(
                        pt_tiles[m][:, base : base + 512], pst[:]
                    )

        with tc.tile_pool(name="psB", bufs=3, space="PSUM") as psB:
            for kb in range(NKB):
                if kb + 3 < NKB:
                    issue_xt()
                if kb == 3:
                    emit_pt_transposes(0)
                elif kb == 5:
                    emit_pt_transposes(1)
                if kb >= 4:
                    # xp rides the late-B DMA shadow in half-chunk pieces
                    # (256 keys each) so transposes interleave promptly
                    for h in range(4 * (kb - 4), 4 * (kb - 3)):
                        jj, hh = h // 2, h % 2
                        nc.sync.dma_start(
                            xp_views[jj][:, 2 * hh : 2 * (hh + 1), :],
                            xp_d[
                                jj * 512 + hh * 256 : jj * 512 + (hh + 1) * 256, :
                            ].rearrange("(kc p) n -> p kc n", p=128),
                        )
                xt_v = xt_views[kb]
                for m in range(NMT):
                    if CAP[m] <= kb:
                        continue
                    ps = psB.tile([128, 512], f32, name="ps_s")
                    for kc in range(KC):
                        nc.tensor.matmul(
                            ps[:],
                            qt_sb[:, kc * RQ + m * 128 : kc * RQ + (m + 1) * 128],
                            xt_v[:, kc, :],
                            start=(kc == 0),
                            stop=(kc == KC - 1),
                        )
                    dst = s_tiles[m][:, kb * 512 : (kb + 1) * 512]
                    if kb == 2 * m:
                        nc.vector.tensor_add(dst, ps[:], maska[:])
                    elif kb == 2 * m + 1:
                        nc.vector.tensor_add(dst, ps[:], maskb[:])
                    else:
                        nc.vector.tensor_copy(dst, ps[:])
                    nc.vector.tensor_reduce(
                        mpart[:, m * NKB + kb : m * NKB + kb + 1],
                        dst,
                        axis=mybir.AxisListType.X,
                        op=mybir.AluOpType.max,
                    )
                    if kb == CAP[m] - 1:
                        # tile m complete: finalize stats, exp, transpose
                        nc.vector.tensor_reduce(
                            negmax[:, m : m + 1],
                            mpart[:, m * NKB : m * NKB + CAP[m]],
                            axis=mybir.AxisListType.X,
                            op=mybir.AluOpType.max,
                            negate=True,
                        )
                        for ch in range(m + 1):
                            p_q = p_pool.tile([128, 1024], f16, name="p_q", tag="pq")
                            nc.scalar.activation(
                                p_q[:],
                                s_tiles[m][:, ch * 1024 : (ch + 1) * 1024],
                                mybir.ActivationFunctionType.Exp,
                                bias=negmax[:, m : m + 1],
                                scale=1.0,
                                accum_out=lq[:, m * 4 + ch : m * 4 + ch + 1],
                            )
                            p_chunks[m].append(p_q)
                        # lsum on the idle Pool engine: keeps this exp-gated
                        # chain out of the DVE FIFO, which must stay clear
                        # for E's PSUM copies (recip is emitted later, in
                        # the E/F loop, for the same reason)
                        nc.gpsimd.tensor_copy(
                            lsum[:, m : m + 1], lq[:, m * 4 : m * 4 + 1]
                        )
                        for ch in range(1, m + 1):
                            nc.gpsimd.tensor_add(
                                lsum[:, m : m + 1],
                                lsum[:, m : m + 1],
                                lq[:, m * 4 + ch : m * 4 + ch + 1],
                            )
        xtstream.release()
        emit_pt_transposes(2)

        # ---- Phases E+F interleaved per row tile -------------------------
        # E(m): o1T[:, m] = sum_k x[k,:]^T P[m,k]^T (ragged contraction),
        # then immediately F(m): out[m] = (o1[m] @ Wov) * recip[m].
        # Keeps the PE stream gapless across the phase boundary; E(m<3)
        # also hides the exp/transpose tail of tile 3.
        with (
            tc.tile_pool(name="psE", bufs=2, space="PSUM") as psE,
            tc.tile_pool(name="psF", bufs=2, space="PSUM") as psF,
            tc.tile_pool(name="outp", bufs=3) as outp,
        ):
            for m in range(NMT):
                if m == 2:
                    # tile 3's transposes: exp(m3) finished long ago by now
                    emit_pt_transposes(3)
                nk = 8 * (m + 1)
                for mtd in range(KC):
                    ps = psE.tile([128, 128], f32, name="ps_av")
                    for kcc in range(nk):
                        nc.tensor.matmul(
                            ps[:],
                            xp_views[kcc // 4][:, kcc % 4, mtd * 128 : (mtd + 1) * 128],
                            pt_views[m][:, kcc, :],
                            start=(kcc == 0),
                            stop=(kcc == nk - 1),
                        )
                    nc.vector.tensor_copy(
                        o1t_sb[:, mtd * RQ + m * 128 : mtd * RQ + (m + 1) * 128],
                        ps[:],
                    )
                nc.vector.reciprocal(recip[:, m : m + 1], lsum[:, m : m + 1])
                for nb in range(2):
                    ps = psF.tile([128, 512], f32, name="ps_o")
                    for kc in range(KC):
                        nc.tensor.matmul(
                            ps[:],
                            o1t_sb[:, kc * RQ + m * 128 : kc * RQ + (m + 1) * 128],
                            wov_tiles[nb][:, kc * 512 : (kc + 1) * 512],
                            start=(kc == 0),
                            stop=(kc == KC - 1),
                        )
                    ob = outp.tile([128, 512], f32, name="ob")
                    nc.vector.tensor_scalar_mul(
                        ob[:], ps[:], recip[:, m : m + 1]
                    )
                    nc.sync.dma_start(
                        out_d[m * 128 : (m + 1) * 128, nb * 512 : (nb + 1) * 512],
                        ob[:],
                    )

        wov_pool.release()
        psT.release()
        qt_pool.release()
        p_pool.release()
        s_pool.release()
        xp_pool.release()
        pt_pool.release()
        o1_pool.release()
        consts.release()

    nc.compile()
    return nc


_NC_CACHE = {}


def _get_nc():
    if "nc" not in _NC_CACHE:
        _NC_CACHE["nc"] = _build_nc()
    return _NC_CACHE["nc"]


def _prep_in_maps(x, Wqk, Wov):
    x = np.ascontiguousarray(np.asarray(x), dtype=np.float32)
    Wqk = np.ascontiguousarray(np.asarray(Wqk), dtype=np.float32)
    Wov = np.ascontiguousarray(np.asarray(Wov), dtype=np.float32)
    x16 = x.astype(np.float16)
    xtp = np.ascontiguousarray(x16.T)  # [D, T] natural key order
    wqk16 = Wqk.astype(np.float16)
    wov16 = Wov.astype(np.float16)

    p = np.arange(128)[:, None]
    col = np.arange(512)[None, :]
    ident = np.eye(128, dtype=np.float16)

    in_maps = []
    for c in range(NCORES):
        rows = np.concatenate(
            [np.arange(128 * (8 * m + c), 128 * (8 * m + c) + 128) for m in range(NMT)]
        )
        xqt = np.ascontiguousarray(x16[rows, :].T)  # [D, RQ]
        if c < 4:
            maska = np.where(col <= 128 * c + p, 0.0, NEG).astype(np.float16)
            maskb = np.full((128, 512), NEG, np.float16)
        else:
            maska = np.zeros((128, 512), np.float16)
            maskb = np.where(col <= 128 * (c - 4) + p, 0.0, NEG).astype(np.float16)
        in_maps.append(
            {
                "xqt": xqt,
                "xtp": xtp,
                "xp": x16,
                "wqk": wqk16,
                "wov": wov16,
                "maska": np.ascontiguousarray(maska),
                "maskb": np.ascontiguousarray(maskb),
                "ident": ident,
            }
        )
    return in_maps


def run(x, Wqk, Wov, **spmd_kwargs):
    """Full pipeline; returns (output [T, D] fp32, BassKernelResults)."""
    import time

    nc = _get_nc()
    in_maps = _prep_in_maps(x, Wqk, Wov)
    try:
        res = run_bass_kernel_spmd(
            nc, in_maps, core_ids=list(range(NCORES)), **spmd_kwargs
        )
    except Exception:
        # a prior crashed execution can leave a core transiently
        # unrecoverable; the runtime resets it — retry once
        time.sleep(10)
        res = run_bass_kernel_spmd(
            nc, in_maps, core_ids=list(range(NCORES)), **spmd_kwargs
        )
    out = np.empty((T, D), np.float32)
    for c in range(NCORES):
        oc = res.results[c]["out"]
        for m in range(NMT):
            g = 8 * m + c
            out[128 * g : 128 * (g + 1), :] = oc[128 * m : 128 * (m + 1), :]
    return out, res


def kernel(x, Wqk, Wov):
    out, _ = run(x, Wqk, Wov)
    return out


# revision 51
# speedup vs baseline: 1.3122x; 1.0442x over previous
"""Causal attention kernel for Trainium2, 8 NeuronCores, sequence-parallel.

Reference computation (T=4096, D=1024, fp32):
    q = x @ Wqk; logits = q @ x.T (causal masked); attn = softmax(logits)
    out = (attn @ x) @ Wov

Causal-balanced sharding: global 128-row query tiles i = 0..31 need
keys 0..128(i+1), i.e. w_i = i//4 + 1 key slots of 512. Core c owns
tiles {c, 8+c, 16+c, 24+c} (local m = 0..3, global g = 8m + c), and the
SPMD program gives local tile m a fixed capacity of 2m+2 key slots
(widths 1024/2048/3072/4096). Every core's needs fit exactly:
  c in 0..3: tile m needs 2m+1 slots -> slot 2m is ragged-diagonal,
             slot 2m+1 is fully masked.
  c in 4..7: tile m needs 2m+2 slots -> slot 2m fully visible,
             slot 2m+1 ragged-diagonal.
Keys stay in NATURAL order and are identical on all cores; only the
query-row selection (xqt columns) and two additive mask tiles differ
per core.  maskA applies at slot 2m, maskB at slot 2m+1, for every m:
  c < 4:  maskA = tri(offset 128c),      maskB = all -60000
  c >= 4: maskA = 0,                     maskB = tri(offset 128(c-4))
This cuts score and AV matmul work to 62.5% of the dense version while
keeping one identical instruction stream on all 8 cores.

Precision: fp16 operands (x, Wqk, Wov, q, attn, o1) with fp32 PSUM
accumulation and fp32 softmax stats; masked-out logits get -60000
(fp16-representable; exp underflows to exactly 0). Host-validated
rel_err ~3e-3 (limit 2e-2).
"""

import sys

sys.path.insert(0, "/opt/trn_rl_repo")

import numpy as np

import concourse.tile as tile
from concourse import bacc, mybir
from concourse.bass_utils import run_bass_kernel_spmd

T = 4096
D = 1024
NCORES = 8
RQ = T // NCORES  # 512 query rows per core
NKB = T // 512  # 8 key slots of 512
KC = D // 128  # 8 contraction chunks
NMT = RQ // 128  # 4 query-row tiles per core
CAP = [2 * m + 2 for m in range(NMT)]  # key-slot capacity per local tile
NEG = -60000.0

f32 = mybir.dt.float32
f16 = mybir.dt.float16


def _build_nc():
    nc = bacc.Bacc(
        "TRN2", target_bir_lowering=False, debug=False, num_devices=NCORES
    )

    xqt_d = nc.dram_tensor("xqt", [D, RQ], f16, kind="ExternalInput").ap()
    xtp_d = nc.dram_tensor("xtp", [D, T], f16, kind="ExternalInput").ap()
    xp_d = nc.dram_tensor("xp", [T, D], f16, kind="ExternalInput").ap()
    wqk_d = nc.dram_tensor("wqk", [D, D], f16, kind="ExternalInput").ap()
    wov_d = nc.dram_tensor("wov", [D, D], f16, kind="ExternalInput").ap()
    maska_d = nc.dram_tensor("maska", [128, 512], f16, kind="ExternalInput").ap()
    maskb_d = nc.dram_tensor("maskb", [128, 512], f16, kind="ExternalInput").ap()
    ident_d = nc.dram_tensor("ident", [128, 128], f16, kind="ExternalInput").ap()
    out_d = nc.dram_tensor("out", [RQ, D], f32, kind="ExternalOutput").ap()

    with tile.TileContext(nc) as tc:
        # stack allocator: allocate in order of decreasing lifetime
        consts = tc.alloc_tile_pool(name="consts", bufs=1)
        o1_pool = tc.alloc_tile_pool(name="o1pool", bufs=1)
        pt_pool = tc.alloc_tile_pool(name="ptpool", bufs=1)
        xp_pool = tc.alloc_tile_pool(name="xppool", bufs=1)
        s_pool = tc.alloc_tile_pool(name="spool", bufs=1)
        p_pool = tc.alloc_tile_pool(name="ppool", bufs=4)
        qt_pool = tc.alloc_tile_pool(name="qtpool", bufs=1)
        # PSUM staging for PE transposes of attn chunks; outlives psB
        psT = tc.alloc_tile_pool(name="psT", bufs=2, space="PSUM")
        # one PSUM pool for phases B, E and F (tags ps_s / ps_av), allocated
        # up front: no pool boundaries between phases, so no cross-phase
        # pool-release waits ever hit the PE queue
        psB = tc.alloc_tile_pool(name="psBEF", bufs=4, space="PSUM")

        # constants: masks + stats scratch
        smalls = consts.tile([128, 64], f32, name="smalls")
        negmax = smalls[:, 0:NMT]
        lsum = smalls[:, 4:8]
        recip = smalls[:, 8:12]
        mpart = smalls[:, 12:44]  # [m * NKB + kb]
        lq = smalls[:, 44:60]  # [m * 4 + ch]
        maska = consts.tile([128, 512], f16, name="maska")
        maskb = consts.tile([128, 512], f16, name="maskb")
        ident = consts.tile([128, 128], f16, name="ident")

        # long-lived big tiles
        o1t_sb = o1_pool.tile([128, KC * RQ], f16, name="o1t_sb")
        pt_tiles = [
            pt_pool.tile([128, 8 * (m + 1) * 128], f16, name=f"pt_m{m}")
            for m in range(NMT)
        ]
        pt_views = [
            ptm.rearrange("p (kcc q) -> p kcc q", kcc=8 * (m + 1))
            for m, ptm in enumerate(pt_tiles)
        ]
        # 8 separate xp tiles (512 keys each) so E's RAW deps are per-chunk
        xp_tiles = [
            xp_pool.tile([128, 4 * D], f16, name=f"xp_{j}") for j in range(8)
        ]
        xp_views = [t.rearrange("p (kc n) -> p kc n", kc=4) for t in xp_tiles]
        s_tiles = [
            s_pool.tile([128, 1024 * (m + 1)], f32, name=f"s_m{m}")
            for m in range(NMT)
        ]
        qt_sb = qt_pool.tile([128, KC * RQ], f16, name="qt_sb")

        # right-side stack: lifetime [A..B] overlaps the left stack's
        # wov pool [A-end..F], so it gets its own stack side
        xtstream = tc.alloc_tile_pool(name="xtstream", bufs=5, side="right")

        xtp_src = xtp_d.rearrange("p (kb n) -> p kb n", kb=NKB)
        xt_views = []

        def issue_xt():
            kb = len(xt_views)
            xt = xtstream.tile([128, KC * 512], f16, name="xt", tag="xt")
            xt_v = xt.rearrange("p (kc n) -> p kc n", kc=KC)
            nc.sync.dma_start(
                xt_v, xtp_src[:, kb, :].rearrange("(kc p) n -> p kc n", p=128)
            )
            xt_views.append(xt_v)

        # warm up the PE clock during the initial DMA wait: back-to-back
        # dummy matmuls on an SBUF scratch region keep the PE busy so the
        # pstate ramp completes before the first real matmul arrives
        nc.gpsimd.memset(smalls[:], 0.0)
        warm = psB.tile([128, 512], f32, name="ps_s", tag="ps_s")
        for _ in range(30):
            nc.tensor.matmul(
                warm[0:64, 0:64], smalls[:, 0:64], smalls[:, 0:64],
                start=True, stop=True,
            )

        # ---- Phase A: qT = (xq @ Wqk)^T -> [D, RQ] fp16 ------------------
        # wqk streamed in 256-col pairs (512B descriptors, no small-desc
        # penalty); each pair feeds two 8-matmul chains so the DMA stream
        # stays ahead of the PE. xt/mask loads ride the spare bandwidth.
        with (
            tc.tile_pool(name="apool", bufs=1) as apool,
            tc.tile_pool(name="wqkstream", bufs=2) as wqkstream,
            tc.tile_pool(name="psA", bufs=2, space="PSUM") as psA,
        ):
            xqt_sb = apool.tile([128, KC * RQ], f16, name="xqt_sb")
            xqt_v = xqt_sb.rearrange("p (kc n) -> p kc n", kc=KC)
            xqt_src = xqt_d.rearrange("(kc p) n -> p kc n", p=128)
            for j in range(4):
                wqk_blk = wqkstream.tile(
                    [128, KC * 256], f16, name="wqk_blk", tag="wq"
                )
                wqk_v = wqk_blk.rearrange("p (kc n) -> p kc n", kc=KC)
                nc.sync.dma_start(
                    wqk_v,
                    wqk_d[:, j * 256 : (j + 1) * 256].rearrange(
                        "(kc p) n -> p kc n", p=128
                    ),
                )
                if j == 0:
                    # quarters so the first chain's kc0 operand lands early
                    for q in range(4):
                        nc.sync.dma_start(
                            xqt_v[:, 2 * q : 2 * (q + 1), :],
                            xqt_src[:, 2 * q : 2 * (q + 1), :],
                        )
                elif j == 1:
                    nc.sync.dma_start(maska, maska_d)
                    nc.sync.dma_start(maskb, maskb_d)
                    nc.sync.dma_start(ident, ident_d)
                elif j == 2:
                    issue_xt()
                else:
                    issue_xt()
                    issue_xt()
                for sub in range(2):
                    mtd = 2 * j + sub
                    ps = psA.tile([128, RQ], f32, name="ps_qt")
                    for kc in range(KC):
                        nc.tensor.matmul(
                            ps[:],
                            wqk_v[:, kc, sub * 128 : (sub + 1) * 128],
                            xqt_v[:, kc, :],
                            start=(kc == 0),
                            stop=(kc == KC - 1),
                        )
                    nc.vector.tensor_copy(
                        qt_sb[:, mtd * RQ : (mtd + 1) * RQ], ps[:]
                    )

        # wov loads reuse the SBUF apool just freed; queued here so they
        # land after xt0-2 but before the rest of the B stream
        wov_pool = tc.alloc_tile_pool(name="wovstream", bufs=1)
        wov_tiles = []
        for nb in range(2):
            wov_blk = wov_pool.tile([128, KC * 512], f16, name=f"wov{nb}")
            wov_tiles.append(wov_blk)
            wv = wov_blk.rearrange("p (kc n) -> p kc n", kc=KC)
            src = wov_d[:, nb * 512 : (nb + 1) * 512].rearrange(
                "(kc p) n -> p kc n", p=128
            )
            # half-size pieces: keeps the exclusive DMA resource fine-grained
            # so latency-critical transposes are not stuck behind them
            nc.sync.dma_start(wv[:, 0:4, :], src[:, 0:4, :])
            nc.sync.dma_start(wv[:, 4:8, :], src[:, 4:8, :])

        # ---- Phase B: ragged scores + fused softmax prep -----------------
        # slot kb serves local tiles m with CAP[m] > kb; masks at slots
        # 2m (maskA) and 2m+1 (maskB); exp+transpose issued per tile as
        # soon as its last slot completes. xp/wov loads ride the late-B
        # DMA shadow, in time for phases E/F.
        p_chunks = [[] for _ in range(NMT)]

        def emit_pt_transposes(m):
            # PE-transpose tile m's attn chunks into pt layout; emitted a
            # couple of key slots after tile m's exp so the PE never waits
            # on the ACT engine. Groups of 4 kcc share one PSUM stage and
            # one DVE copy.
            for ch in range(m + 1):
                p_q = p_chunks[m][ch]
                for g in range(2):
                    pst = psT.tile([128, 512], f16, name="pst", tag="pst")
                    for q4 in range(4):
                        col = (g * 4 + q4) * 128
                        nc.tensor.transpose(
                            pst[:, q4 * 128 : (q4 + 1) * 128],
                            p_q[:, col : col + 128],
                            ident[:],
                        )
                    base = (ch * 8 + g * 4) * 128
                    nc.vector.tensor_copy(
                        pt_tiles[m][:, base : base + 512], pst[:]
                    )

        if True:
            for kb in range(NKB):
                if kb + 3 < NKB:
                    issue_xt()
                if kb == 3:
                    emit_pt_transposes(0)
                elif kb == 5:
                    emit_pt_transposes(1)
                if kb >= 4:
                    # xp rides the late-B DMA shadow in half-chunk pieces
                    # (256 keys each) so transposes interleave promptly
                    for h in range(4 * (kb - 4), 4 * (kb - 3)):
                        jj, hh = h // 2, h % 2
                        nc.sync.dma_start(
                            xp_views[jj][:, 2 * hh : 2 * (hh + 1), :],
                            xp_d[
                                jj * 512 + hh * 256 : jj * 512 + (hh + 1) * 256, :
                            ].rearrange("(kc p) n -> p kc n", p=128),
                        )
                xt_v = xt_views[kb]
                for m in range(NMT):
                    if CAP[m] <= kb:
                        continue
                    ps = psB.tile([128, 512], f32, name="ps_s", tag="ps_s")
                    for kc in range(KC):
                        nc.tensor.matmul(
                            ps[:],
                            qt_sb[:, kc * RQ + m * 128 : kc * RQ + (m + 1) * 128],
                            xt_v[:, kc, :],
                            start=(kc == 0),
                            stop=(kc == KC - 1),
                        )
                    dst = s_tiles[m][:, kb * 512 : (kb + 1) * 512]
                    if kb == 2 * m:
                        nc.vector.tensor_add(dst, ps[:], maska[:])
                    elif kb == 2 * m + 1:
                        nc.vector.tensor_add(dst, ps[:], maskb[:])
                    else:
                        nc.vector.tensor_copy(dst, ps[:])
                    nc.vector.tensor_reduce(
                        mpart[:, m * NKB + kb : m * NKB + kb + 1],
                        dst,
                        axis=mybir.AxisListType.X,
                        op=mybir.AluOpType.max,
                    )
                    if kb == CAP[m] - 1:
                        # tile m complete: finalize stats, exp, transpose
                        nc.vector.tensor_reduce(
                            negmax[:, m : m + 1],
                            mpart[:, m * NKB : m * NKB + CAP[m]],
                            axis=mybir.AxisListType.X,
                            op=mybir.AluOpType.max,
                            negate=True,
                        )
                        for ch in range(m + 1):
                            p_q = p_pool.tile([128, 1024], f16, name="p_q", tag="pq")
                            nc.scalar.activation(
                                p_q[:],
                                s_tiles[m][:, ch * 1024 : (ch + 1) * 1024],
                                mybir.ActivationFunctionType.Exp,
                                bias=negmax[:, m : m + 1],
                                scale=1.0,
                                accum_out=lq[:, m * 4 + ch : m * 4 + ch + 1],
                            )
                            p_chunks[m].append(p_q)
                        # lsum on the idle Pool engine: keeps this exp-gated
                        # chain out of the DVE FIFO, which must stay clear
                        # for E's PSUM copies (recip is emitted later, in
                        # the E/F loop, for the same reason)
                        nc.gpsimd.tensor_copy(
                            lsum[:, m : m + 1], lq[:, m * 4 : m * 4 + 1]
                        )
                        for ch in range(1, m + 1):
                            nc.gpsimd.tensor_add(
                                lsum[:, m : m + 1],
                                lsum[:, m : m + 1],
                                lq[:, m * 4 + ch : m * 4 + ch + 1],
                            )
        xtstream.release()
        emit_pt_transposes(2)

        # ---- Phases E+F interleaved per row tile -------------------------
        # E(m): o1T[:, m] = sum_k x[k,:]^T P[m,k]^T (ragged contraction),
        # then immediately F(m): out[m] = (o1[m] @ Wov) * recip[m].
        # Keeps the PE stream gapless across the phase boundary; E(m<3)
        # also hides the exp/transpose tail of tile 3.
        with tc.tile_pool(name="outp", bufs=3) as outp:
            for m in range(NMT):
                if m == 2:
                    # tile 3's transposes: exp(m3) finished long ago by now
                    emit_pt_transposes(3)
                nk = 8 * (m + 1)
                o1t_v = o1t_sb.rearrange("p (kc n) -> p kc n", kc=KC)
                for half in range(2):
                    # 4 mtd chains share one bank-sized PSUM tile (disjoint
                    # 128-col regions), then drain with a single DVE copy
                    ps = psB.tile([128, 512], f32, name="ps_s", tag="ps_s")
                    for ml in range(4):
                        mtd = half * 4 + ml
                        for kcc in range(nk):
                            nc.tensor.matmul(
                                ps[:, ml * 128 : (ml + 1) * 128],
                                xp_views[kcc // 4][
                                    :, kcc % 4, mtd * 128 : (mtd + 1) * 128
                                ],
                                pt_views[m][:, kcc, :],
                                start=(kcc == 0),
                                stop=(kcc == nk - 1),
                            )
                    nc.vector.tensor_copy(
                        o1t_v[
                            :, half * 4 : (half + 1) * 4, m * 128 : (m + 1) * 128
                        ],
                        ps.rearrange("p (c n) -> p c n", c=4),
                    )
                nc.vector.reciprocal(recip[:, m : m + 1], lsum[:, m : m + 1])
                for nb in range(2):
                    ps = psB.tile([128, 512], f32, name="ps_s", tag="ps_s")
                    for kc in range(KC):
                        nc.tensor.matmul(
                            ps[:],
                            o1t_sb[:, kc * RQ + m * 128 : kc * RQ + (m + 1) * 128],
                            wov_tiles[nb][:, kc * 512 : (kc + 1) * 512],
                            start=(kc == 0),
                            stop=(kc == KC - 1),
                        )
                    ob = outp.tile([128, 512], f32, name="ob")
                    nc.vector.tensor_scalar_mul(
                        ob[:], ps[:], recip[:, m : m + 1]
                    )
                    nc.sync.dma_start(
                        out_d[m * 128 : (m + 1) * 128, nb * 512 : (nb + 1) * 512],
                        ob[:],
                    )

        wov_pool.release()
        psB.release()
        psT.release()
        qt_pool.release()
        p_pool.release()
        s_pool.release()
        xp_pool.release()
        pt_pool.release()
        o1_pool.release()
        consts.release()

    nc.compile()
    return nc


_NC_CACHE = {}


def _get_nc():
    if "nc" not in _NC_CACHE:
        _NC_CACHE["nc"] = _build_nc()
    return _NC_CACHE["nc"]


def _prep_in_maps(x, Wqk, Wov):
    x = np.ascontiguousarray(np.asarray(x), dtype=np.float32)
    Wqk = np.ascontiguousarray(np.asarray(Wqk), dtype=np.float32)
    Wov = np.ascontiguousarray(np.asarray(Wov), dtype=np.float32)
    x16 = x.astype(np.float16)
    xtp = np.ascontiguousarray(x16.T)  # [D, T] natural key order
    wqk16 = Wqk.astype(np.float16)
    wov16 = Wov.astype(np.float16)

    p = np.arange(128)[:, None]
    col = np.arange(512)[None, :]
    ident = np.eye(128, dtype=np.float16)

    in_maps = []
    for c in range(NCORES):
        rows = np.concatenate(
            [np.arange(128 * (8 * m + c), 128 * (8 * m + c) + 128) for m in range(NMT)]
        )
        xqt = np.ascontiguousarray(x16[rows, :].T)  # [D, RQ]
        if c < 4:
            maska = np.where(col <= 128 * c + p, 0.0, NEG).astype(np.float16)
            maskb = np.full((128, 512), NEG, np.float16)
        else:
            maska = np.zeros((128, 512), np.float16)
            maskb = np.where(col <= 128 * (c - 4) + p, 0.0, NEG).astype(np.float16)
        in_maps.append(
            {
                "xqt": xqt,
                "xtp": xtp,
                "xp": x16,
                "wqk": wqk16,
                "wov": wov16,
                "maska": np.ascontiguousarray(maska),
                "maskb": np.ascontiguousarray(maskb),
                "ident": ident,
            }
        )
    return in_maps


def run(x, Wqk, Wov, **spmd_kwargs):
    """Full pipeline; returns (output [T, D] fp32, BassKernelResults)."""
    import time

    nc = _get_nc()
    in_maps = _prep_in_maps(x, Wqk, Wov)
    try:
        res = run_bass_kernel_spmd(
            nc, in_maps, core_ids=list(range(NCORES)), **spmd_kwargs
        )
    except Exception:
        # a prior crashed execution can leave a core transiently
        # unrecoverable; the runtime resets it — retry once
        time.sleep(10)
        res = run_bass_kernel_spmd(
            nc, in_maps, core_ids=list(range(NCORES)), **spmd_kwargs
        )
    out = np.empty((T, D), np.float32)
    for c in range(NCORES):
        oc = res.results[c]["out"]
        for m in range(NMT):
            g = 8 * m + c
            out[128 * g : 128 * (g + 1), :] = oc[128 * m : 128 * (m + 1), :]
    return out, res


def kernel(x, Wqk, Wov):
    out, _ = run(x, Wqk, Wov)
    return out


# revision 52
# speedup vs baseline: 1.3350x; 1.0174x over previous
"""Causal attention kernel for Trainium2, 8 NeuronCores, sequence-parallel.

Reference computation (T=4096, D=1024, fp32):
    q = x @ Wqk; logits = q @ x.T (causal masked); attn = softmax(logits)
    out = (attn @ x) @ Wov

Causal-balanced sharding: global 128-row query tiles i = 0..31 need
keys 0..128(i+1), i.e. w_i = i//4 + 1 key slots of 512. Core c owns
tiles {c, 8+c, 16+c, 24+c} (local m = 0..3, global g = 8m + c), and the
SPMD program gives local tile m a fixed capacity of 2m+2 key slots
(widths 1024/2048/3072/4096). Every core's needs fit exactly:
  c in 0..3: tile m needs 2m+1 slots -> slot 2m is ragged-diagonal,
             slot 2m+1 is fully masked.
  c in 4..7: tile m needs 2m+2 slots -> slot 2m fully visible,
             slot 2m+1 ragged-diagonal.
Keys stay in NATURAL order and are identical on all cores; only the
query-row selection (xqt columns) and two additive mask tiles differ
per core.  maskA applies at slot 2m, maskB at slot 2m+1, for every m:
  c < 4:  maskA = tri(offset 128c),      maskB = all -60000
  c >= 4: maskA = 0,                     maskB = tri(offset 128(c-4))
This cuts score and AV matmul work to 62.5% of the dense version while
keeping one identical instruction stream on all 8 cores.

Precision: fp16 operands (x, Wqk, Wov, q, attn, o1) with fp32 PSUM
accumulation and fp32 softmax stats; masked-out logits get -60000
(fp16-representable; exp underflows to exactly 0). Host-validated
rel_err ~3e-3 (limit 2e-2).
"""

import sys

sys.path.insert(0, "/opt/trn_rl_repo")

import numpy as np

import concourse.tile as tile
from concourse import bacc, mybir
from concourse.bass_utils import run_bass_kernel_spmd

T = 4096
D = 1024
NCORES = 8
RQ = T // NCORES  # 512 query rows per core
NKB = T // 512  # 8 key slots of 512
KC = D // 128  # 8 contraction chunks
NMT = RQ // 128  # 4 query-row tiles per core
CAP = [2 * m + 2 for m in range(NMT)]  # key-slot capacity per local tile
NEG = -60000.0

f32 = mybir.dt.float32
f16 = mybir.dt.float16


def _build_nc():
    nc = bacc.Bacc(
        "TRN2", target_bir_lowering=False, debug=False, num_devices=NCORES
    )

    xqt_d = nc.dram_tensor("xqt", [D, RQ], f16, kind="ExternalInput").ap()
    xtp_d = nc.dram_tensor("xtp", [D, T], f16, kind="ExternalInput").ap()
    xp_d = nc.dram_tensor("xp", [T, D], f16, kind="ExternalInput").ap()
    wqk_d = nc.dram_tensor("wqk", [D, D], f16, kind="ExternalInput").ap()
    wov_d = nc.dram_tensor("wov", [D, D], f16, kind="ExternalInput").ap()
    maska_d = nc.dram_tensor("maska", [128, 512], f16, kind="ExternalInput").ap()
    maskb_d = nc.dram_tensor("maskb", [128, 512], f16, kind="ExternalInput").ap()
    ident_d = nc.dram_tensor("ident", [128, 128], f16, kind="ExternalInput").ap()
    out_d = nc.dram_tensor("out", [RQ, D], f32, kind="ExternalOutput").ap()

    with tile.TileContext(nc) as tc:
        # stack allocator: allocate in order of decreasing lifetime
        consts = tc.alloc_tile_pool(name="consts", bufs=1)
        o1_pool = tc.alloc_tile_pool(name="o1pool", bufs=1)
        pt_pool = tc.alloc_tile_pool(name="ptpool", bufs=1)
        xp_pool = tc.alloc_tile_pool(name="xppool", bufs=1)
        s_pool = tc.alloc_tile_pool(name="spool", bufs=1)
        p_pool = tc.alloc_tile_pool(name="ppool", bufs=4)
        qt_pool = tc.alloc_tile_pool(name="qtpool", bufs=1)
        # PSUM staging for PE transposes of attn chunks; outlives psB
        psT = tc.alloc_tile_pool(name="psT", bufs=2, space="PSUM")
        # one PSUM pool for phases B, E and F (tags ps_s / ps_av), allocated
        # up front: no pool boundaries between phases, so no cross-phase
        # pool-release waits ever hit the PE queue
        psB = tc.alloc_tile_pool(name="psBEF", bufs=4, space="PSUM")

        # constants: masks + stats scratch
        smalls = consts.tile([128, 64], f32, name="smalls")
        negmax = smalls[:, 0:NMT]
        lsum = smalls[:, 4:8]
        recip = smalls[:, 8:12]
        mpart = smalls[:, 12:44]  # [m * NKB + kb]
        lq = smalls[:, 44:60]  # [m * 4 + ch]
        maska = consts.tile([128, 512], f16, name="maska")
        maskb = consts.tile([128, 512], f16, name="maskb")
        ident = consts.tile([128, 128], f16, name="ident")

        # long-lived big tiles
        o1t_sb = o1_pool.tile([128, KC * RQ], f16, name="o1t_sb")
        pt_tiles = [
            pt_pool.tile([128, 8 * (m + 1) * 128], f16, name=f"pt_m{m}")
            for m in range(NMT)
        ]
        pt_views = [
            ptm.rearrange("p (kcc q) -> p kcc q", kcc=8 * (m + 1))
            for m, ptm in enumerate(pt_tiles)
        ]
        # 8 separate xp tiles (512 keys each) so E's RAW deps are per-chunk
        xp_tiles = [
            xp_pool.tile([128, 4 * D], f16, name=f"xp_{j}") for j in range(8)
        ]
        xp_views = [t.rearrange("p (kc n) -> p kc n", kc=4) for t in xp_tiles]
        s_tiles = [
            s_pool.tile([128, 1024 * (m + 1)], f32, name=f"s_m{m}")
            for m in range(NMT)
        ]
        qt_sb = qt_pool.tile([128, KC * RQ], f16, name="qt_sb")

        # right-side stack: lifetime [A..B] overlaps the left stack's
        # wov pool [A-end..F], so it gets its own stack side
        xtstream = tc.alloc_tile_pool(name="xtstream", bufs=5, side="right")

        xtp_src = xtp_d.rearrange("p (kb n) -> p kb n", kb=NKB)
        xt_views = []

        def issue_xt():
            kb = len(xt_views)
            xt = xtstream.tile([128, KC * 512], f16, name="xt", tag="xt")
            xt_v = xt.rearrange("p (kc n) -> p kc n", kc=KC)
            nc.sync.dma_start(
                xt_v, xtp_src[:, kb, :].rearrange("(kc p) n -> p kc n", p=128)
            )
            xt_views.append(xt_v)

        # warm up the PE clock during the initial DMA wait: back-to-back
        # dummy matmuls on an SBUF scratch region keep the PE busy so the
        # pstate ramp completes before the first real matmul arrives
        nc.gpsimd.memset(smalls[:], 0.0)
        warm = psB.tile([128, 512], f32, name="ps_s", tag="ps_s")
        for _ in range(20):
            nc.tensor.matmul(
                warm[0:64, 0:64], smalls[:, 0:64], smalls[:, 0:64],
                start=True, stop=True,
            )

        # ---- Phase A: qT = (xq @ Wqk)^T -> [D, RQ] fp16 ------------------
        # wqk streamed in 256-col pairs (512B descriptors, no small-desc
        # penalty); each pair feeds two 8-matmul chains so the DMA stream
        # stays ahead of the PE. xt/mask loads ride the spare bandwidth.
        with (
            tc.tile_pool(name="apool", bufs=1) as apool,
            tc.tile_pool(name="wqkstream", bufs=2) as wqkstream,
            tc.tile_pool(name="psA", bufs=2, space="PSUM") as psA,
        ):
            xqt_sb = apool.tile([128, KC * RQ], f16, name="xqt_sb")
            xqt_v = xqt_sb.rearrange("p (kc n) -> p kc n", kc=KC)
            xqt_src = xqt_d.rearrange("(kc p) n -> p kc n", p=128)
            for j in range(4):
                wqk_blk = wqkstream.tile(
                    [128, KC * 256], f16, name="wqk_blk", tag="wq"
                )
                wqk_v = wqk_blk.rearrange("p (kc n) -> p kc n", kc=KC)
                nc.sync.dma_start(
                    wqk_v,
                    wqk_d[:, j * 256 : (j + 1) * 256].rearrange(
                        "(kc p) n -> p kc n", p=128
                    ),
                )
                if j == 0:
                    # quarters so the first chain's kc0 operand lands early
                    for q in range(4):
                        nc.sync.dma_start(
                            xqt_v[:, 2 * q : 2 * (q + 1), :],
                            xqt_src[:, 2 * q : 2 * (q + 1), :],
                        )
                elif j == 1:
                    nc.sync.dma_start(maska, maska_d)
                    nc.sync.dma_start(maskb, maskb_d)
                    nc.sync.dma_start(ident, ident_d)
                elif j == 2:
                    issue_xt()
                else:
                    issue_xt()
                    issue_xt()
                for sub in range(2):
                    mtd = 2 * j + sub
                    ps = psA.tile([128, RQ], f32, name="ps_qt")
                    for kc in range(KC):
                        nc.tensor.matmul(
                            ps[:],
                            wqk_v[:, kc, sub * 128 : (sub + 1) * 128],
                            xqt_v[:, kc, :],
                            start=(kc == 0),
                            stop=(kc == KC - 1),
                        )
                    nc.vector.tensor_copy(
                        qt_sb[:, mtd * RQ : (mtd + 1) * RQ], ps[:]
                    )

        # wov loads reuse the SBUF apool just freed; queued here so they
        # land after xt0-2 but before the rest of the B stream
        wov_pool = tc.alloc_tile_pool(name="wovstream", bufs=1)
        wov_tiles = []
        for nb in range(2):
            wov_blk = wov_pool.tile([128, KC * 512], f16, name=f"wov{nb}")
            wov_tiles.append(wov_blk)
            wv = wov_blk.rearrange("p (kc n) -> p kc n", kc=KC)
            src = wov_d[:, nb * 512 : (nb + 1) * 512].rearrange(
                "(kc p) n -> p kc n", p=128
            )
            # half-size pieces: keeps the exclusive DMA resource fine-grained
            # so latency-critical transposes are not stuck behind them
            nc.sync.dma_start(wv[:, 0:4, :], src[:, 0:4, :])
            nc.sync.dma_start(wv[:, 4:8, :], src[:, 4:8, :])

        # ---- Phase B: ragged scores + fused softmax prep -----------------
        # slot kb serves local tiles m with CAP[m] > kb; masks at slots
        # 2m (maskA) and 2m+1 (maskB); exp+transpose issued per tile as
        # soon as its last slot completes. xp/wov loads ride the late-B
        # DMA shadow, in time for phases E/F.
        p_chunks = [[] for _ in range(NMT)]

        def emit_pt_transposes(m):
            # PE-transpose tile m's attn chunks into pt layout; emitted a
            # couple of key slots after tile m's exp so the PE never waits
            # on the ACT engine. Groups of 4 kcc share one PSUM stage and
            # one DVE copy.
            for ch in range(m + 1):
                p_q = p_chunks[m][ch]
                for g in range(2):
                    pst = psT.tile([128, 512], f16, name="pst", tag="pst")
                    for q4 in range(4):
                        col = (g * 4 + q4) * 128
                        nc.tensor.transpose(
                            pst[:, q4 * 128 : (q4 + 1) * 128],
                            p_q[:, col : col + 128],
                            ident[:],
                        )
                    base = (ch * 8 + g * 4) * 128
                    nc.vector.tensor_copy(
                        pt_tiles[m][:, base : base + 512], pst[:]
                    )

        if True:
            for kb in range(NKB):
                if kb + 3 < NKB:
                    issue_xt()
                if kb == 3:
                    emit_pt_transposes(0)
                elif kb == 5:
                    emit_pt_transposes(1)
                if kb >= 4:
                    # xp rides the late-B DMA shadow in half-chunk pieces
                    # (256 keys each) so transposes interleave promptly
                    for h in range(4 * (kb - 4), 4 * (kb - 3)):
                        jj, hh = h // 2, h % 2
                        nc.sync.dma_start(
                            xp_views[jj][:, 2 * hh : 2 * (hh + 1), :],
                            xp_d[
                                jj * 512 + hh * 256 : jj * 512 + (hh + 1) * 256, :
                            ].rearrange("(kc p) n -> p kc n", p=128),
                        )
                xt_v = xt_views[kb]
                for m in range(NMT):
                    if CAP[m] <= kb:
                        continue
                    ps = psB.tile([128, 512], f32, name="ps_s", tag="ps_s")
                    for kc in range(KC):
                        nc.tensor.matmul(
                            ps[:],
                            qt_sb[:, kc * RQ + m * 128 : kc * RQ + (m + 1) * 128],
                            xt_v[:, kc, :],
                            start=(kc == 0),
                            stop=(kc == KC - 1),
                        )
                    dst = s_tiles[m][:, kb * 512 : (kb + 1) * 512]
                    if kb == 2 * m:
                        nc.vector.tensor_add(dst, ps[:], maska[:])
                    elif kb == 2 * m + 1:
                        nc.vector.tensor_add(dst, ps[:], maskb[:])
                    else:
                        nc.vector.tensor_copy(dst, ps[:])
                    nc.vector.tensor_reduce(
                        mpart[:, m * NKB + kb : m * NKB + kb + 1],
                        dst,
                        axis=mybir.AxisListType.X,
                        op=mybir.AluOpType.max,
                    )
                    if kb == CAP[m] - 1:
                        # tile m complete: finalize stats, exp, transpose
                        nc.vector.tensor_reduce(
                            negmax[:, m : m + 1],
                            mpart[:, m * NKB : m * NKB + CAP[m]],
                            axis=mybir.AxisListType.X,
                            op=mybir.AluOpType.max,
                            negate=True,
                        )
                        for ch in range(m + 1):
                            p_q = p_pool.tile([128, 1024], f16, name="p_q", tag="pq")
                            nc.scalar.activation(
                                p_q[:],
                                s_tiles[m][:, ch * 1024 : (ch + 1) * 1024],
                                mybir.ActivationFunctionType.Exp,
                                bias=negmax[:, m : m + 1],
                                scale=1.0,
                                accum_out=lq[:, m * 4 + ch : m * 4 + ch + 1],
                            )
                            p_chunks[m].append(p_q)
                        # lsum on the idle Pool engine: keeps this exp-gated
                        # chain out of the DVE FIFO, which must stay clear
                        # for E's PSUM copies (recip is emitted later, in
                        # the E/F loop, for the same reason)
                        nc.gpsimd.tensor_copy(
                            lsum[:, m : m + 1], lq[:, m * 4 : m * 4 + 1]
                        )
                        for ch in range(1, m + 1):
                            nc.gpsimd.tensor_add(
                                lsum[:, m : m + 1],
                                lsum[:, m : m + 1],
                                lq[:, m * 4 + ch : m * 4 + ch + 1],
                            )
        xtstream.release()
        emit_pt_transposes(2)

        # ---- Phases E+F interleaved per row tile -------------------------
        # E(m): o1T[:, m] = sum_k x[k,:]^T P[m,k]^T (ragged contraction),
        # then immediately F(m): out[m] = (o1[m] @ Wov) * recip[m].
        # Keeps the PE stream gapless across the phase boundary; E(m<3)
        # also hides the exp/transpose tail of tile 3.
        with tc.tile_pool(name="outp", bufs=3) as outp:
            for m in range(NMT):
                if m == 2:
                    # tile 3's transposes: exp(m3) finished long ago by now
                    emit_pt_transposes(3)
                nk = 8 * (m + 1)
                o1t_v = o1t_sb.rearrange("p (kc n) -> p kc n", kc=KC)
                for half in range(2):
                    # 4 mtd chains share one bank-sized PSUM tile (disjoint
                    # 128-col regions), then drain with a single DVE copy
                    ps = psB.tile([128, 512], f32, name="ps_s", tag="ps_s")
                    for ml in range(4):
                        mtd = half * 4 + ml
                        for kcc in range(nk):
                            nc.tensor.matmul(
                                ps[:, ml * 128 : (ml + 1) * 128],
                                xp_views[kcc // 4][
                                    :, kcc % 4, mtd * 128 : (mtd + 1) * 128
                                ],
                                pt_views[m][:, kcc, :],
                                start=(kcc == 0),
                                stop=(kcc == nk - 1),
                            )
                    nc.vector.tensor_copy(
                        o1t_v[
                            :, half * 4 : (half + 1) * 4, m * 128 : (m + 1) * 128
                        ],
                        ps.rearrange("p (c n) -> p c n", c=4),
                    )
                nc.vector.reciprocal(recip[:, m : m + 1], lsum[:, m : m + 1])
                for nb in range(2):
                    ps = psB.tile([128, 512], f32, name="ps_s", tag="ps_s")
                    for kc in range(KC):
                        nc.tensor.matmul(
                            ps[:],
                            o1t_sb[:, kc * RQ + m * 128 : kc * RQ + (m + 1) * 128],
                            wov_tiles[nb][:, kc * 512 : (kc + 1) * 512],
                            start=(kc == 0),
                            stop=(kc == KC - 1),
                        )
                    ob = outp.tile([128, 512], f32, name="ob")
                    nc.vector.tensor_scalar_mul(
                        ob[:], ps[:], recip[:, m : m + 1]
                    )
                    nc.sync.dma_start(
                        out_d[m * 128 : (m + 1) * 128, nb * 512 : (nb + 1) * 512],
                        ob[:],
                    )

        wov_pool.release()
        psB.release()
        psT.release()
        qt_pool.release()
        p_pool.release()
        s_pool.release()
        xp_pool.release()
        pt_pool.release()
        o1_pool.release()
        consts.release()

    nc.compile()
    return nc


_NC_CACHE = {}


def _get_nc():
    if "nc" not in _NC_CACHE:
        _NC_CACHE["nc"] = _build_nc()
    return _NC_CACHE["nc"]


def _prep_in_maps(x, Wqk, Wov):
    x = np.ascontiguousarray(np.asarray(x), dtype=np.float32)
    Wqk = np.ascontiguousarray(np.asarray(Wqk), dtype=np.float32)
    Wov = np.ascontiguousarray(np.asarray(Wov), dtype=np.float32)
    x16 = x.astype(np.float16)
    xtp = np.ascontiguousarray(x16.T)  # [D, T] natural key order
    wqk16 = Wqk.astype(np.float16)
    wov16 = Wov.astype(np.float16)

    p = np.arange(128)[:, None]
    col = np.arange(512)[None, :]
    ident = np.eye(128, dtype=np.float16)

    in_maps = []
    for c in range(NCORES):
        rows = np.concatenate(
            [np.arange(128 * (8 * m + c), 128 * (8 * m + c) + 128) for m in range(NMT)]
        )
        xqt = np.ascontiguousarray(x16[rows, :].T)  # [D, RQ]
        if c < 4:
            maska = np.where(col <= 128 * c + p, 0.0, NEG).astype(np.float16)
            maskb = np.full((128, 512), NEG, np.float16)
        else:
            maska = np.zeros((128, 512), np.float16)
            maskb = np.where(col <= 128 * (c - 4) + p, 0.0, NEG).astype(np.float16)
        in_maps.append(
            {
                "xqt": xqt,
                "xtp": xtp,
                "xp": x16,
                "wqk": wqk16,
                "wov": wov16,
                "maska": np.ascontiguousarray(maska),
                "maskb": np.ascontiguousarray(maskb),
                "ident": ident,
            }
        )
    return in_maps


def run(x, Wqk, Wov, **spmd_kwargs):
    """Full pipeline; returns (output [T, D] fp32, BassKernelResults)."""
    import time

    nc = _get_nc()
    in_maps = _prep_in_maps(x, Wqk, Wov)
    try:
        res = run_bass_kernel_spmd(
            nc, in_maps, core_ids=list(range(NCORES)), **spmd_kwargs
        )
    except Exception:
        # a prior crashed execution can leave a core transiently
        # unrecoverable; the runtime resets it — retry once
        time.sleep(10)
        res = run_bass_kernel_spmd(
            nc, in_maps, core_ids=list(range(NCORES)), **spmd_kwargs
        )
    out = np.empty((T, D), np.float32)
    for c in range(NCORES):
        oc = res.results[c]["out"]
        for m in range(NMT):
            g = 8 * m + c
            out[128 * g : 128 * (g + 1), :] = oc[128 * m : 128 * (m + 1), :]
    return out, res


def kernel(x, Wqk, Wov):
    out, _ = run(x, Wqk, Wov)
    return out


# revision 53
# speedup vs baseline: 1.3358x; 1.0006x over previous
"""Causal attention kernel for Trainium2, 8 NeuronCores, sequence-parallel.

Reference computation (T=4096, D=1024, fp32):
    q = x @ Wqk; logits = q @ x.T (causal masked); attn = softmax(logits)
    out = (attn @ x) @ Wov

Causal-balanced sharding: global 128-row query tiles i = 0..31 need
keys 0..128(i+1), i.e. w_i = i//4 + 1 key slots of 512. Core c owns
tiles {c, 8+c, 16+c, 24+c} (local m = 0..3, global g = 8m + c), and the
SPMD program gives local tile m a fixed capacity of 2m+2 key slots
(widths 1024/2048/3072/4096). Every core's needs fit exactly:
  c in 0..3: tile m needs 2m+1 slots -> slot 2m is ragged-diagonal,
             slot 2m+1 is fully masked.
  c in 4..7: tile m needs 2m+2 slots -> slot 2m fully visible,
             slot 2m+1 ragged-diagonal.
Keys stay in NATURAL order and are identical on all cores; only the
query-row selection (xqt columns) and two additive mask tiles differ
per core.  maskA applies at slot 2m, maskB at slot 2m+1, for every m:
  c < 4:  maskA = tri(offset 128c),      maskB = all -60000
  c >= 4: maskA = 0,                     maskB = tri(offset 128(c-4))
This cuts score and AV matmul work to 62.5% of the dense version while
keeping one identical instruction stream on all 8 cores.

Precision: fp16 operands (x, Wqk, Wov, q, attn, o1) with fp32 PSUM
accumulation and fp32 softmax stats; masked-out logits get -60000
(fp16-representable; exp underflows to exactly 0). Host-validated
rel_err ~3e-3 (limit 2e-2).
"""

import sys

sys.path.insert(0, "/opt/trn_rl_repo")

import numpy as np

import concourse.tile as tile
from concourse import bacc, mybir
from concourse.bass_utils import run_bass_kernel_spmd

T = 4096
D = 1024
NCORES = 8
RQ = T // NCORES  # 512 query rows per core
NKB = T // 512  # 8 key slots of 512
KC = D // 128  # 8 contraction chunks
NMT = RQ // 128  # 4 query-row tiles per core
CAP = [2 * m + 2 for m in range(NMT)]  # key-slot capacity per local tile
NEG = -60000.0

f32 = mybir.dt.float32
f16 = mybir.dt.float16


def _build_nc():
    nc = bacc.Bacc(
        "TRN2", target_bir_lowering=False, debug=False, num_devices=NCORES
    )

    xqt_d = nc.dram_tensor("xqt", [D, RQ], f16, kind="ExternalInput").ap()
    xtp_d = nc.dram_tensor("xtp", [D, T], f16, kind="ExternalInput").ap()
    xp_d = nc.dram_tensor("xp", [T, D], f16, kind="ExternalInput").ap()
    wqk_d = nc.dram_tensor("wqk", [D, D], f16, kind="ExternalInput").ap()
    wov_d = nc.dram_tensor("wov", [D, D], f16, kind="ExternalInput").ap()
    maska_d = nc.dram_tensor("maska", [128, 512], f16, kind="ExternalInput").ap()
    maskb_d = nc.dram_tensor("maskb", [128, 512], f16, kind="ExternalInput").ap()
    ident_d = nc.dram_tensor("ident", [128, 128], f16, kind="ExternalInput").ap()
    out_d = nc.dram_tensor("out", [RQ, D], f32, kind="ExternalOutput").ap()

    with tile.TileContext(nc) as tc:
        # stack allocator: allocate in order of decreasing lifetime
        consts = tc.alloc_tile_pool(name="consts", bufs=1)
        o1_pool = tc.alloc_tile_pool(name="o1pool", bufs=1)
        pt_pool = tc.alloc_tile_pool(name="ptpool", bufs=1)
        xp_pool = tc.alloc_tile_pool(name="xppool", bufs=1)
        s_pool = tc.alloc_tile_pool(name="spool", bufs=1)
        p_pool = tc.alloc_tile_pool(name="ppool", bufs=4)
        qt_pool = tc.alloc_tile_pool(name="qtpool", bufs=1)
        # PSUM staging for PE transposes of attn chunks; outlives psB
        psT = tc.alloc_tile_pool(name="psT", bufs=2, space="PSUM")
        # one PSUM pool for phases B, E and F (tags ps_s / ps_av), allocated
        # up front: no pool boundaries between phases, so no cross-phase
        # pool-release waits ever hit the PE queue
        psB = tc.alloc_tile_pool(name="psBEF", bufs=4, space="PSUM")

        # constants: masks + stats scratch
        smalls = consts.tile([128, 64], f32, name="smalls")
        negmax = smalls[:, 0:NMT]
        lsum = smalls[:, 4:8]
        recip = smalls[:, 8:12]
        mpart = smalls[:, 12:44]  # [m * NKB + kb]
        lq = smalls[:, 44:60]  # [m * 4 + ch]
        maska = consts.tile([128, 512], f16, name="maska")
        maskb = consts.tile([128, 512], f16, name="maskb")
        ident = consts.tile([128, 128], f16, name="ident")

        # long-lived big tiles
        o1t_sb = o1_pool.tile([128, KC * RQ], f16, name="o1t_sb")
        pt_tiles = [
            pt_pool.tile([128, 8 * (m + 1) * 128], f16, name=f"pt_m{m}")
            for m in range(NMT)
        ]
        pt_views = [
            ptm.rearrange("p (kcc q) -> p kcc q", kcc=8 * (m + 1))
            for m, ptm in enumerate(pt_tiles)
        ]
        # 8 separate xp tiles (512 keys each) so E's RAW deps are per-chunk
        xp_tiles = [
            xp_pool.tile([128, 4 * D], f16, name=f"xp_{j}") for j in range(8)
        ]
        xp_views = [t.rearrange("p (kc n) -> p kc n", kc=4) for t in xp_tiles]
        s_tiles = [
            s_pool.tile([128, 1024 * (m + 1)], f32, name=f"s_m{m}")
            for m in range(NMT)
        ]
        qt_sb = qt_pool.tile([128, KC * RQ], f16, name="qt_sb")

        # right-side stack: lifetime [A..B] overlaps the left stack's
        # wov pool [A-end..F], so it gets its own stack side
        xtstream = tc.alloc_tile_pool(name="xtstream", bufs=5, side="right")

        xtp_src = xtp_d.rearrange("p (kb n) -> p kb n", kb=NKB)
        xt_views = []

        def issue_xt():
            kb = len(xt_views)
            xt = xtstream.tile([128, KC * 512], f16, name="xt", tag="xt")
            xt_v = xt.rearrange("p (kc n) -> p kc n", kc=KC)
            nc.sync.dma_start(
                xt_v, xtp_src[:, kb, :].rearrange("(kc p) n -> p kc n", p=128)
            )
            xt_views.append(xt_v)

        # warm up the PE clock during the initial DMA wait: back-to-back
        # dummy matmuls on an SBUF scratch region keep the PE busy so the
        # pstate ramp completes before the first real matmul arrives
        nc.gpsimd.memset(smalls[:], 0.0)
        warm = psB.tile([128, 512], f32, name="ps_s", tag="ps_s")
        for _ in range(16):
            nc.tensor.matmul(
                warm[0:64, 0:64], smalls[:, 0:64], smalls[:, 0:64],
                start=True, stop=True,
            )

        # ---- Phase A: qT = (xq @ Wqk)^T -> [D, RQ] fp16 ------------------
        # wqk streamed in 256-col pairs (512B descriptors, no small-desc
        # penalty); each pair feeds two 8-matmul chains so the DMA stream
        # stays ahead of the PE. xt/mask loads ride the spare bandwidth.
        with (
            tc.tile_pool(name="apool", bufs=1) as apool,
            tc.tile_pool(name="wqkstream", bufs=2) as wqkstream,
            tc.tile_pool(name="psA", bufs=2, space="PSUM") as psA,
        ):
            xqt_sb = apool.tile([128, KC * RQ], f16, name="xqt_sb")
            xqt_v = xqt_sb.rearrange("p (kc n) -> p kc n", kc=KC)
            xqt_src = xqt_d.rearrange("(kc p) n -> p kc n", p=128)
            for j in range(4):
                wqk_blk = wqkstream.tile(
                    [128, KC * 256], f16, name="wqk_blk", tag="wq"
                )
                wqk_v = wqk_blk.rearrange("p (kc n) -> p kc n", kc=KC)
                nc.sync.dma_start(
                    wqk_v,
                    wqk_d[:, j * 256 : (j + 1) * 256].rearrange(
                        "(kc p) n -> p kc n", p=128
                    ),
                )
                if j == 0:
                    # quarters so the first chain's kc0 operand lands early
                    for q in range(4):
                        nc.sync.dma_start(
                            xqt_v[:, 2 * q : 2 * (q + 1), :],
                            xqt_src[:, 2 * q : 2 * (q + 1), :],
                        )
                elif j == 1:
                    nc.sync.dma_start(maska, maska_d)
                    nc.sync.dma_start(maskb, maskb_d)
                    nc.sync.dma_start(ident, ident_d)
                elif j == 2:
                    issue_xt()
                else:
                    issue_xt()
                    issue_xt()
                for sub in range(2):
                    mtd = 2 * j + sub
                    ps = psA.tile([128, RQ], f32, name="ps_qt")
                    for kc in range(KC):
                        nc.tensor.matmul(
                            ps[:],
                            wqk_v[:, kc, sub * 128 : (sub + 1) * 128],
                            xqt_v[:, kc, :],
                            start=(kc == 0),
                            stop=(kc == KC - 1),
                        )
                    nc.vector.tensor_copy(
                        qt_sb[:, mtd * RQ : (mtd + 1) * RQ], ps[:]
                    )

        # wov loads reuse the SBUF apool just freed; queued here so they
        # land after xt0-2 but before the rest of the B stream
        wov_pool = tc.alloc_tile_pool(name="wovstream", bufs=1)
        wov_tiles = []
        for nb in range(2):
            wov_blk = wov_pool.tile([128, KC * 512], f16, name=f"wov{nb}")
            wov_tiles.append(wov_blk)
            wv = wov_blk.rearrange("p (kc n) -> p kc n", kc=KC)
            src = wov_d[:, nb * 512 : (nb + 1) * 512].rearrange(
                "(kc p) n -> p kc n", p=128
            )
            # half-size pieces: keeps the exclusive DMA resource fine-grained
            # so latency-critical transposes are not stuck behind them
            nc.sync.dma_start(wv[:, 0:4, :], src[:, 0:4, :])
            nc.sync.dma_start(wv[:, 4:8, :], src[:, 4:8, :])

        # ---- Phase B: ragged scores + fused softmax prep -----------------
        # slot kb serves local tiles m with CAP[m] > kb; masks at slots
        # 2m (maskA) and 2m+1 (maskB); exp+transpose issued per tile as
        # soon as its last slot completes. xp/wov loads ride the late-B
        # DMA shadow, in time for phases E/F.
        p_chunks = [[] for _ in range(NMT)]

        def emit_pt_transposes(m):
            # PE-transpose tile m's attn chunks into pt layout; emitted a
            # couple of key slots after tile m's exp so the PE never waits
            # on the ACT engine. Groups of 4 kcc share one PSUM stage and
            # one DVE copy.
            for ch in range(m + 1):
                p_q = p_chunks[m][ch]
                for g in range(2):
                    pst = psT.tile([128, 512], f16, name="pst", tag="pst")
                    for q4 in range(4):
                        col = (g * 4 + q4) * 128
                        nc.tensor.transpose(
                            pst[:, q4 * 128 : (q4 + 1) * 128],
                            p_q[:, col : col + 128],
                            ident[:],
                        )
                    base = (ch * 8 + g * 4) * 128
                    nc.vector.tensor_copy(
                        pt_tiles[m][:, base : base + 512], pst[:]
                    )

        if True:
            for kb in range(NKB):
                if kb + 3 < NKB:
                    issue_xt()
                if kb == 3:
                    emit_pt_transposes(0)
                elif kb == 5:
                    emit_pt_transposes(1)
                if kb >= 4:
                    # xp rides the late-B DMA shadow in half-chunk pieces
                    # (256 keys each) so transposes interleave promptly
                    for h in range(4 * (kb - 4), 4 * (kb - 3)):
                        jj, hh = h // 2, h % 2
                        nc.sync.dma_start(
                            xp_views[jj][:, 2 * hh : 2 * (hh + 1), :],
                            xp_d[
                                jj * 512 + hh * 256 : jj * 512 + (hh + 1) * 256, :
                            ].rearrange("(kc p) n -> p kc n", p=128),
                        )
                xt_v = xt_views[kb]
                for m in range(NMT):
                    if CAP[m] <= kb:
                        continue
                    ps = psB.tile([128, 512], f32, name="ps_s", tag="ps_s")
                    for kc in range(KC):
                        nc.tensor.matmul(
                            ps[:],
                            qt_sb[:, kc * RQ + m * 128 : kc * RQ + (m + 1) * 128],
                            xt_v[:, kc, :],
                            start=(kc == 0),
                            stop=(kc == KC - 1),
                        )
                    dst = s_tiles[m][:, kb * 512 : (kb + 1) * 512]
                    if kb == 2 * m:
                        nc.vector.tensor_add(dst, ps[:], maska[:])
                    elif kb == 2 * m + 1:
                        nc.vector.tensor_add(dst, ps[:], maskb[:])
                    else:
                        nc.vector.tensor_copy(dst, ps[:])
                    nc.vector.tensor_reduce(
                        mpart[:, m * NKB + kb : m * NKB + kb + 1],
                        dst,
                        axis=mybir.AxisListType.X,
                        op=mybir.AluOpType.max,
                    )
                    if kb == CAP[m] - 1:
                        # tile m complete: finalize stats, exp, transpose
                        nc.vector.tensor_reduce(
                            negmax[:, m : m + 1],
                            mpart[:, m * NKB : m * NKB + CAP[m]],
                            axis=mybir.AxisListType.X,
                            op=mybir.AluOpType.max,
                            negate=True,
                        )
                        for ch in range(m + 1):
                            p_q = p_pool.tile([128, 1024], f16, name="p_q", tag="pq")
                            nc.scalar.activation(
                                p_q[:],
                                s_tiles[m][:, ch * 1024 : (ch + 1) * 1024],
                                mybir.ActivationFunctionType.Exp,
                                bias=negmax[:, m : m + 1],
                                scale=1.0,
                                accum_out=lq[:, m * 4 + ch : m * 4 + ch + 1],
                            )
                            p_chunks[m].append(p_q)
                        # lsum on the idle Pool engine: keeps this exp-gated
                        # chain out of the DVE FIFO, which must stay clear
                        # for E's PSUM copies (recip is emitted later, in
                        # the E/F loop, for the same reason)
                        nc.gpsimd.tensor_copy(
                            lsum[:, m : m + 1], lq[:, m * 4 : m * 4 + 1]
                        )
                        for ch in range(1, m + 1):
                            nc.gpsimd.tensor_add(
                                lsum[:, m : m + 1],
                                lsum[:, m : m + 1],
                                lq[:, m * 4 + ch : m * 4 + ch + 1],
                            )
        xtstream.release()
        emit_pt_transposes(2)

        # ---- Phases E+F interleaved per row tile -------------------------
        # E(m): o1T[:, m] = sum_k x[k,:]^T P[m,k]^T (ragged contraction),
        # then immediately F(m): out[m] = (o1[m] @ Wov) * recip[m].
        # Keeps the PE stream gapless across the phase boundary; E(m<3)
        # also hides the exp/transpose tail of tile 3.
        with tc.tile_pool(name="outp", bufs=3) as outp:
            for m in range(NMT):
                if m == 2:
                    # tile 3's transposes: exp(m3) finished long ago by now
                    emit_pt_transposes(3)
                nk = 8 * (m + 1)
                o1t_v = o1t_sb.rearrange("p (kc n) -> p kc n", kc=KC)
                for half in range(2):
                    # 4 mtd chains share one bank-sized PSUM tile (disjoint
                    # 128-col regions), then drain with a single DVE copy
                    ps = psB.tile([128, 512], f32, name="ps_s", tag="ps_s")
                    for ml in range(4):
                        mtd = half * 4 + ml
                        for kcc in range(nk):
                            nc.tensor.matmul(
                                ps[:, ml * 128 : (ml + 1) * 128],
                                xp_views[kcc // 4][
                                    :, kcc % 4, mtd * 128 : (mtd + 1) * 128
                                ],
                                pt_views[m][:, kcc, :],
                                start=(kcc == 0),
                                stop=(kcc == nk - 1),
                            )
                    nc.vector.tensor_copy(
                        o1t_v[
                            :, half * 4 : (half + 1) * 4, m * 128 : (m + 1) * 128
                        ],
                        ps.rearrange("p (c n) -> p c n", c=4),
                    )
                nc.vector.reciprocal(recip[:, m : m + 1], lsum[:, m : m + 1])
                for nb in range(2):
                    ps = psB.tile([128, 512], f32, name="ps_s", tag="ps_s")
                    for kc in range(KC):
                        nc.tensor.matmul(
                            ps[:],
                            o1t_sb[:, kc * RQ + m * 128 : kc * RQ + (m + 1) * 128],
                            wov_tiles[nb][:, kc * 512 : (kc + 1) * 512],
                            start=(kc == 0),
                            stop=(kc == KC - 1),
                        )
                    ob = outp.tile([128, 512], f32, name="ob")
                    nc.vector.tensor_scalar_mul(
                        ob[:], ps[:], recip[:, m : m + 1]
                    )
                    nc.sync.dma_start(
                        out_d[m * 128 : (m + 1) * 128, nb * 512 : (nb + 1) * 512],
                        ob[:],
                    )

        wov_pool.release()
        psB.release()
        psT.release()
        qt_pool.release()
        p_pool.release()
        s_pool.release()
        xp_pool.release()
        pt_pool.release()
        o1_pool.release()
        consts.release()

    nc.compile()
    return nc


_NC_CACHE = {}


def _get_nc():
    if "nc" not in _NC_CACHE:
        _NC_CACHE["nc"] = _build_nc()
    return _NC_CACHE["nc"]


def _prep_in_maps(x, Wqk, Wov):
    x = np.ascontiguousarray(np.asarray(x), dtype=np.float32)
    Wqk = np.ascontiguousarray(np.asarray(Wqk), dtype=np.float32)
    Wov = np.ascontiguousarray(np.asarray(Wov), dtype=np.float32)
    x16 = x.astype(np.float16)
    xtp = np.ascontiguousarray(x16.T)  # [D, T] natural key order
    wqk16 = Wqk.astype(np.float16)
    wov16 = Wov.astype(np.float16)

    p = np.arange(128)[:, None]
    col = np.arange(512)[None, :]
    ident = np.eye(128, dtype=np.float16)

    in_maps = []
    for c in range(NCORES):
        rows = np.concatenate(
            [np.arange(128 * (8 * m + c), 128 * (8 * m + c) + 128) for m in range(NMT)]
        )
        xqt = np.ascontiguousarray(x16[rows, :].T)  # [D, RQ]
        if c < 4:
            maska = np.where(col <= 128 * c + p, 0.0, NEG).astype(np.float16)
            maskb = np.full((128, 512), NEG, np.float16)
        else:
            maska = np.zeros((128, 512), np.float16)
            maskb = np.where(col <= 128 * (c - 4) + p, 0.0, NEG).astype(np.float16)
        in_maps.append(
            {
                "xqt": xqt,
                "xtp": xtp,
                "xp": x16,
                "wqk": wqk16,
                "wov": wov16,
                "maska": np.ascontiguousarray(maska),
                "maskb": np.ascontiguousarray(maskb),
                "ident": ident,
            }
        )
    return in_maps


def run(x, Wqk, Wov, **spmd_kwargs):
    """Full pipeline; returns (output [T, D] fp32, BassKernelResults)."""
    import time

    nc = _get_nc()
    in_maps = _prep_in_maps(x, Wqk, Wov)
    try:
        res = run_bass_kernel_spmd(
            nc, in_maps, core_ids=list(range(NCORES)), **spmd_kwargs
        )
    except Exception:
        # a prior crashed execution can leave a core transiently
        # unrecoverable; the runtime resets it — retry once
        time.sleep(10)
        res = run_bass_kernel_spmd(
            nc, in_maps, core_ids=list(range(NCORES)), **spmd_kwargs
        )
    out = np.empty((T, D), np.float32)
    for c in range(NCORES):
        oc = res.results[c]["out"]
        for m in range(NMT):
            g = 8 * m + c
            out[128 * g : 128 * (g + 1), :] = oc[128 * m : 128 * (m + 1), :]
    return out, res


def kernel(x, Wqk, Wov):
    out, _ = run(x, Wqk, Wov)
    return out


# revision 55
# speedup vs baseline: 1.3427x; 1.0052x over previous
"""Causal attention kernel for Trainium2, 8 NeuronCores, sequence-parallel.

Reference computation (T=4096, D=1024, fp32):
    q = x @ Wqk; logits = q @ x.T (causal masked); attn = softmax(logits)
    out = (attn @ x) @ Wov

Causal-balanced sharding: global 128-row query tiles i = 0..31 need
keys 0..128(i+1), i.e. w_i = i//4 + 1 key slots of 512. Core c owns
tiles {c, 8+c, 16+c, 24+c} (local m = 0..3, global g = 8m + c), and the
SPMD program gives local tile m a fixed capacity of 2m+2 key slots
(widths 1024/2048/3072/4096). Every core's needs fit exactly:
  c in 0..3: tile m needs 2m+1 slots -> slot 2m is ragged-diagonal,
             slot 2m+1 is fully masked.
  c in 4..7: tile m needs 2m+2 slots -> slot 2m fully visible,
             slot 2m+1 ragged-diagonal.
Keys stay in NATURAL order and are identical on all cores; only the
query-row selection (xqt columns) and two additive mask tiles differ
per core.  maskA applies at slot 2m, maskB at slot 2m+1, for every m:
  c < 4:  maskA = tri(offset 128c),      maskB = all -60000
  c >= 4: maskA = 0,                     maskB = tri(offset 128(c-4))
This cuts score and AV matmul work to 62.5% of the dense version while
keeping one identical instruction stream on all 8 cores.

Precision: fp16 operands (x, Wqk, Wov, q, attn, o1) with fp32 PSUM
accumulation and fp32 softmax stats; masked-out logits get -60000
(fp16-representable; exp underflows to exactly 0). Host-validated
rel_err ~3e-3 (limit 2e-2).
"""

import sys

sys.path.insert(0, "/opt/trn_rl_repo")

import numpy as np

import concourse.tile as tile
from concourse import bacc, mybir
from concourse.bass_utils import run_bass_kernel_spmd

T = 4096
D = 1024
NCORES = 8
RQ = T // NCORES  # 512 query rows per core
NKB = T // 512  # 8 key slots of 512
KC = D // 128  # 8 contraction chunks
NMT = RQ // 128  # 4 query-row tiles per core
CAP = [2 * m + 2 for m in range(NMT)]  # key-slot capacity per local tile
NEG = -60000.0

f32 = mybir.dt.float32
f16 = mybir.dt.float16


def _build_nc():
    nc = bacc.Bacc(
        "TRN2", target_bir_lowering=False, debug=False, num_devices=NCORES
    )

    xqt_d = nc.dram_tensor("xqt", [D, RQ], f16, kind="ExternalInput").ap()
    xtp_d = nc.dram_tensor("xtp", [D, T], f16, kind="ExternalInput").ap()
    xp_d = nc.dram_tensor("xp", [T, D], f16, kind="ExternalInput").ap()
    wqk_d = nc.dram_tensor("wqk", [D, D], f16, kind="ExternalInput").ap()
    wov_d = nc.dram_tensor("wov", [D, D], f16, kind="ExternalInput").ap()
    maska_d = nc.dram_tensor("maska", [128, 512], f16, kind="ExternalInput").ap()
    maskb_d = nc.dram_tensor("maskb", [128, 512], f16, kind="ExternalInput").ap()
    ident_d = nc.dram_tensor("ident", [128, 128], f16, kind="ExternalInput").ap()
    out_d = nc.dram_tensor("out", [RQ, D], f32, kind="ExternalOutput").ap()

    with tile.TileContext(nc) as tc:
        # stack allocator: allocate in order of decreasing lifetime
        consts = tc.alloc_tile_pool(name="consts", bufs=1)
        o1_pool = tc.alloc_tile_pool(name="o1pool", bufs=1)
        pt_pool = tc.alloc_tile_pool(name="ptpool", bufs=1)
        xp_pool = tc.alloc_tile_pool(name="xppool", bufs=1)
        s_pool = tc.alloc_tile_pool(name="spool", bufs=1)
        p_pool = tc.alloc_tile_pool(name="ppool", bufs=4)
        qt_pool = tc.alloc_tile_pool(name="qtpool", bufs=1)
        # PSUM staging for PE transposes of attn chunks; outlives psB
        psT = tc.alloc_tile_pool(name="psT", bufs=2, space="PSUM")
        # one PSUM pool for phases B, E and F (tags ps_s / ps_av), allocated
        # up front: no pool boundaries between phases, so no cross-phase
        # pool-release waits ever hit the PE queue
        psB = tc.alloc_tile_pool(name="psBEF", bufs=4, space="PSUM")

        # constants: masks + stats scratch
        smalls = consts.tile([128, 64], f32, name="smalls")
        negmax = smalls[:, 0:NMT]
        lsum = smalls[:, 4:8]
        recip = smalls[:, 8:12]
        mpart = smalls[:, 12:44]  # [m * NKB + kb]
        lq = smalls[:, 44:60]  # [m * 4 + ch]
        maska = consts.tile([128, 512], f16, name="maska")
        maskb = consts.tile([128, 512], f16, name="maskb")
        ident = consts.tile([128, 128], f16, name="ident")

        # long-lived big tiles
        o1t_sb = o1_pool.tile([128, KC * RQ], f16, name="o1t_sb")
        pt_tiles = [
            pt_pool.tile([128, 8 * (m + 1) * 128], f16, name=f"pt_m{m}")
            for m in range(NMT)
        ]
        pt_views = [
            ptm.rearrange("p (kcc q) -> p kcc q", kcc=8 * (m + 1))
            for m, ptm in enumerate(pt_tiles)
        ]
        # 8 separate xp tiles (512 keys each) so E's RAW deps are per-chunk
        xp_tiles = [
            xp_pool.tile([128, 4 * D], f16, name=f"xp_{j}") for j in range(8)
        ]
        xp_views = [t.rearrange("p (kc n) -> p kc n", kc=4) for t in xp_tiles]
        s_tiles = [
            s_pool.tile([128, 1024 * (m + 1)], f32, name=f"s_m{m}")
            for m in range(NMT)
        ]
        qt_sb = qt_pool.tile([128, KC * RQ], f16, name="qt_sb")

        # right-side stack: lifetime [A..B] overlaps the left stack's
        # wov pool [A-end..F], so it gets its own stack side
        xtstream = tc.alloc_tile_pool(name="xtstream", bufs=5, side="right")

        xtp_src = xtp_d.rearrange("p (kb n) -> p kb n", kb=NKB)
        xt_views = []

        def issue_xt():
            kb = len(xt_views)
            xt = xtstream.tile([128, KC * 512], f16, name="xt", tag="xt")
            xt_v = xt.rearrange("p (kc n) -> p kc n", kc=KC)
            nc.sync.dma_start(
                xt_v, xtp_src[:, kb, :].rearrange("(kc p) n -> p kc n", p=128)
            )
            xt_views.append(xt_v)

        # warm up the PE clock during the initial DMA wait: back-to-back
        # dummy matmuls on an SBUF scratch region keep the PE busy so the
        # pstate ramp completes before the first real matmul arrives
        nc.gpsimd.memset(smalls[:], 0.0)
        warm = psB.tile([128, 512], f32, name="ps_s", tag="ps_s")
        for _ in range(18):
            nc.tensor.matmul(
                warm[0:64, 0:64], smalls[:, 0:64], smalls[:, 0:64],
                start=True, stop=True,
            )

        # ---- Phase A: qT = (xq @ Wqk)^T -> [D, RQ] fp16 ------------------
        # wqk streamed in 256-col pairs (512B descriptors, no small-desc
        # penalty); each pair feeds two 8-matmul chains so the DMA stream
        # stays ahead of the PE. xt/mask loads ride the spare bandwidth.
        with (
            tc.tile_pool(name="apool", bufs=1) as apool,
            tc.tile_pool(name="wqkstream", bufs=2) as wqkstream,
            tc.tile_pool(name="psA", bufs=2, space="PSUM") as psA,
        ):
            xqt_sb = apool.tile([128, KC * RQ], f16, name="xqt_sb")
            xqt_v = xqt_sb.rearrange("p (kc n) -> p kc n", kc=KC)
            xqt_src = xqt_d.rearrange("(kc p) n -> p kc n", p=128)
            for j in range(4):
                wqk_blk = wqkstream.tile(
                    [128, KC * 256], f16, name="wqk_blk", tag="wq"
                )
                wqk_v = wqk_blk.rearrange("p (kc n) -> p kc n", kc=KC)
                nc.sync.dma_start(
                    wqk_v,
                    wqk_d[:, j * 256 : (j + 1) * 256].rearrange(
                        "(kc p) n -> p kc n", p=128
                    ),
                )
                if j == 0:
                    # quarters so the first chain's kc0 operand lands early
                    for q in range(4):
                        nc.sync.dma_start(
                            xqt_v[:, 2 * q : 2 * (q + 1), :],
                            xqt_src[:, 2 * q : 2 * (q + 1), :],
                        )
                elif j == 1:
                    nc.sync.dma_start(maska, maska_d)
                    nc.sync.dma_start(maskb, maskb_d)
                    nc.sync.dma_start(ident, ident_d)
                elif j == 2:
                    issue_xt()
                else:
                    issue_xt()
                    issue_xt()
                for sub in range(2):
                    mtd = 2 * j + sub
                    ps = psA.tile([128, RQ], f32, name="ps_qt")
                    for kc in range(KC):
                        nc.tensor.matmul(
                            ps[:],
                            wqk_v[:, kc, sub * 128 : (sub + 1) * 128],
                            xqt_v[:, kc, :],
                            start=(kc == 0),
                            stop=(kc == KC - 1),
                        )
                    nc.vector.tensor_copy(
                        qt_sb[:, mtd * RQ : (mtd + 1) * RQ], ps[:]
                    )

        # wov loads reuse the SBUF apool just freed; queued here so they
        # land after xt0-2 but before the rest of the B stream
        wov_pool = tc.alloc_tile_pool(name="wovstream", bufs=1)
        wov_tiles = []
        for nb in range(2):
            wov_blk = wov_pool.tile([128, KC * 512], f16, name=f"wov{nb}")
            wov_tiles.append(wov_blk)
            wv = wov_blk.rearrange("p (kc n) -> p kc n", kc=KC)
            src = wov_d[:, nb * 512 : (nb + 1) * 512].rearrange(
                "(kc p) n -> p kc n", p=128
            )
            # half-size pieces: keeps the exclusive DMA resource fine-grained
            # so latency-critical transposes are not stuck behind them
            nc.sync.dma_start(wv[:, 0:4, :], src[:, 0:4, :])
            nc.sync.dma_start(wv[:, 4:8, :], src[:, 4:8, :])

        # ---- Phase B: ragged scores + fused softmax prep -----------------
        # slot kb serves local tiles m with CAP[m] > kb; masks at slots
        # 2m (maskA) and 2m+1 (maskB); exp+transpose issued per tile as
        # soon as its last slot completes. xp/wov loads ride the late-B
        # DMA shadow, in time for phases E/F.
        p_chunks = [[] for _ in range(NMT)]

        def emit_pt_transposes(m):
            # PE-transpose tile m's attn chunks into pt layout; emitted a
            # couple of key slots after tile m's exp so the PE never waits
            # on the ACT engine. Groups of 4 kcc share one PSUM stage and
            # one DVE copy.
            for ch in range(m + 1):
                p_q = p_chunks[m][ch]
                for g in range(2):
                    pst = psT.tile([128, 512], f16, name="pst", tag="pst")
                    for q4 in range(4):
                        col = (g * 4 + q4) * 128
                        nc.tensor.transpose(
                            pst[:, q4 * 128 : (q4 + 1) * 128],
                            p_q[:, col : col + 128],
                            ident[:],
                        )
                    base = (ch * 8 + g * 4) * 128
                    nc.vector.tensor_copy(
                        pt_tiles[m][:, base : base + 512], pst[:]
                    )

        if True:
            for kb in range(NKB):
                if kb + 3 < NKB:
                    issue_xt()
                if kb == 3:
                    emit_pt_transposes(0)
                elif kb == 5:
                    emit_pt_transposes(1)
                if kb >= 4:
                    # xp rides the late-B DMA shadow in half-chunk pieces
                    # (256 keys each) so transposes interleave promptly
                    for h in range(4 * (kb - 4), 4 * (kb - 3)):
                        jj, hh = h // 2, h % 2
                        nc.sync.dma_start(
                            xp_views[jj][:, 2 * hh : 2 * (hh + 1), :],
                            xp_d[
                                jj * 512 + hh * 256 : jj * 512 + (hh + 1) * 256, :
                            ].rearrange("(kc p) n -> p kc n", p=128),
                        )
                xt_v = xt_views[kb]
                for m in range(NMT):
                    if CAP[m] <= kb:
                        continue
                    ps = psB.tile([128, 512], f32, name="ps_s", tag="ps_s")
                    for kc in range(KC):
                        nc.tensor.matmul(
                            ps[:],
                            qt_sb[:, kc * RQ + m * 128 : kc * RQ + (m + 1) * 128],
                            xt_v[:, kc, :],
                            start=(kc == 0),
                            stop=(kc == KC - 1),
                        )
                    dst = s_tiles[m][:, kb * 512 : (kb + 1) * 512]
                    if kb == 2 * m:
                        nc.vector.tensor_add(dst, ps[:], maska[:])
                    elif kb == 2 * m + 1:
                        nc.vector.tensor_add(dst, ps[:], maskb[:])
                    else:
                        nc.vector.tensor_copy(dst, ps[:])
                    nc.vector.tensor_reduce(
                        mpart[:, m * NKB + kb : m * NKB + kb + 1],
                        dst,
                        axis=mybir.AxisListType.X,
                        op=mybir.AluOpType.max,
                    )
                    if kb == CAP[m] - 1:
                        # tile m complete: finalize stats, exp, transpose
                        nc.vector.tensor_reduce(
                            negmax[:, m : m + 1],
                            mpart[:, m * NKB : m * NKB + CAP[m]],
                            axis=mybir.AxisListType.X,
                            op=mybir.AluOpType.max,
                            negate=True,
                        )
                        for ch in range(m + 1):
                            p_q = p_pool.tile([128, 1024], f16, name="p_q", tag="pq")
                            nc.scalar.activation(
                                p_q[:],
                                s_tiles[m][:, ch * 1024 : (ch + 1) * 1024],
                                mybir.ActivationFunctionType.Exp,
                                bias=negmax[:, m : m + 1],
                                scale=1.0,
                                accum_out=lq[:, m * 4 + ch : m * 4 + ch + 1],
                            )
                            p_chunks[m].append(p_q)
                        # lsum on the idle Pool engine: keeps this exp-gated
                        # chain out of the DVE FIFO, which must stay clear
                        # for E's PSUM copies (recip is emitted later, in
                        # the E/F loop, for the same reason)
                        nc.gpsimd.tensor_copy(
                            lsum[:, m : m + 1], lq[:, m * 4 : m * 4 + 1]
                        )
                        for ch in range(1, m + 1):
                            nc.gpsimd.tensor_add(
                                lsum[:, m : m + 1],
                                lsum[:, m : m + 1],
                                lq[:, m * 4 + ch : m * 4 + ch + 1],
                            )
        xtstream.release()
        emit_pt_transposes(2)

        # ---- Phases E+F interleaved per row tile -------------------------
        # E(m): o1T[:, m] = sum_k x[k,:]^T P[m,k]^T (ragged contraction),
        # then immediately F(m): out[m] = (o1[m] @ Wov) * recip[m].
        # Keeps the PE stream gapless across the phase boundary; E(m<3)
        # also hides the exp/transpose tail of tile 3.
        with tc.tile_pool(name="outp", bufs=3) as outp:
            for m in range(NMT):
                if m == 2:
                    # tile 3's transposes: exp(m3) finished long ago by now
                    emit_pt_transposes(3)
                nk = 8 * (m + 1)
                o1t_v = o1t_sb.rearrange("p (kc n) -> p kc n", kc=KC)
                for half in range(2):
                    # 4 mtd chains share one bank-sized PSUM tile (disjoint
                    # 128-col regions), then drain with a single DVE copy
                    ps = psB.tile([128, 512], f32, name="ps_s", tag="ps_s")
                    for ml in range(4):
                        mtd = half * 4 + ml
                        for kcc in range(nk):
                            nc.tensor.matmul(
                                ps[:, ml * 128 : (ml + 1) * 128],
                                xp_views[kcc // 4][
                                    :, kcc % 4, mtd * 128 : (mtd + 1) * 128
                                ],
                                pt_views[m][:, kcc, :],
                                start=(kcc == 0),
                                stop=(kcc == nk - 1),
                            )
                    nc.vector.tensor_copy(
                        o1t_v[
                            :, half * 4 : (half + 1) * 4, m * 128 : (m + 1) * 128
                        ],
                        ps.rearrange("p (c n) -> p c n", c=4),
                    )
                nc.vector.reciprocal(recip[:, m : m + 1], lsum[:, m : m + 1])
                for nb in range(2):
                    # the very last output is computed in two half-width
                    # chains so the final mul+DMA latency overlaps the PE
                    # tail instead of trailing it
                    pieces = 2 if (m == NMT - 1 and nb == 1) else 1
                    w = 512 // pieces
                    for hh in range(pieces):
                        ps = psB.tile([128, 512], f32, name="ps_s", tag="ps_s")
                        for kc in range(KC):
                            nc.tensor.matmul(
                                ps[:, 0:w],
                                o1t_sb[
                                    :, kc * RQ + m * 128 : kc * RQ + (m + 1) * 128
                                ],
                                wov_tiles[nb][
                                    :, kc * 512 + hh * w : kc * 512 + (hh + 1) * w
                                ],
                                start=(kc == 0),
                                stop=(kc == KC - 1),
                            )
                        ob = outp.tile([128, 512], f32, name="ob")
                        nc.vector.tensor_scalar_mul(
                            ob[:, 0:w], ps[:, 0:w], recip[:, m : m + 1]
                        )
                        nc.sync.dma_start(
                            out_d[
                                m * 128 : (m + 1) * 128,
                                nb * 512 + hh * w : nb * 512 + (hh + 1) * w,
                            ],
                            ob[:, 0:w],
                        )

        wov_pool.release()
        psB.release()
        psT.release()
        qt_pool.release()
        p_pool.release()
        s_pool.release()
        xp_pool.release()
        pt_pool.release()
        o1_pool.release()
        consts.release()

    nc.compile()
    return nc


_NC_CACHE = {}


def _get_nc():
    if "nc" not in _NC_CACHE:
        _NC_CACHE["nc"] = _build_nc()
    return _NC_CACHE["nc"]


def _prep_in_maps(x, Wqk, Wov):
    x = np.ascontiguousarray(np.asarray(x), dtype=np.float32)
    Wqk = np.ascontiguousarray(np.asarray(Wqk), dtype=np.float32)
    Wov = np.ascontiguousarray(np.asarray(Wov), dtype=np.float32)
    x16 = x.astype(np.float16)
    xtp = np.ascontiguousarray(x16.T)  # [D, T] natural key order
    wqk16 = Wqk.astype(np.float16)
    wov16 = Wov.astype(np.float16)

    p = np.arange(128)[:, None]
    col = np.arange(512)[None, :]
    ident = np.eye(128, dtype=np.float16)

    in_maps = []
    for c in range(NCORES):
        rows = np.concatenate(
            [np.arange(128 * (8 * m + c), 128 * (8 * m + c) + 128) for m in range(NMT)]
        )
        xqt = np.ascontiguousarray(x16[rows, :].T)  # [D, RQ]
        if c < 4:
            maska = np.where(col <= 128 * c + p, 0.0, NEG).astype(np.float16)
            maskb = np.full((128, 512), NEG, np.float16)
        else:
            maska = np.zeros((128, 512), np.float16)
            maskb = np.where(col <= 128 * (c - 4) + p, 0.0, NEG).astype(np.float16)
        in_maps.append(
            {
                "xqt": xqt,
                "xtp": xtp,
                "xp": x16,
                "wqk": wqk16,
                "wov": wov16,
                "maska": np.ascontiguousarray(maska),
                "maskb": np.ascontiguousarray(maskb),
                "ident": ident,
            }
        )
    return in_maps


def run(x, Wqk, Wov, **spmd_kwargs):
    """Full pipeline; returns (output [T, D] fp32, BassKernelResults)."""
    import time

    nc = _get_nc()
    in_maps = _prep_in_maps(x, Wqk, Wov)
    try:
        res = run_bass_kernel_spmd(
            nc, in_maps, core_ids=list(range(NCORES)), **spmd_kwargs
        )
    except Exception:
        # a prior crashed execution can leave a core transiently
        # unrecoverable; the runtime resets it — retry once
        time.sleep(10)
        res = run_bass_kernel_spmd(
            nc, in_maps, core_ids=list(range(NCORES)), **spmd_kwargs
        )
    out = np.empty((T, D), np.float32)
    for c in range(NCORES):
        oc = res.results[c]["out"]
        for m in range(NMT):
            g = 8 * m + c
            out[128 * g : 128 * (g + 1), :] = oc[128 * m : 128 * (m + 1), :]
    return out, res


def kernel(x, Wqk, Wov):
    out, _ = run(x, Wqk, Wov)
    return out
